# revision 28
# baseline (speedup 1.0000x reference)
"""Multi-head attention (B=4, S=2048, D=512, H=8) on 8 trn2 NeuronCores.

Sharding: core c handles batch b=c//2, head-group g=c%2 (4 heads, 256 of the
512 projection dims). Each core runs the full fused pipeline for its four
heads - QKV projection, scores^T = K_h Q_h^T, exp (softmax numerator),
attn @ V with a folded ones-column producing the softmax denominators,
normalization, and its partial output projection y^T = Wo_slice^T.T @ O^T.
The host sums the two partial y^T per batch and adds the output bias.

Key performance structure (vs the v1 kernel):
- Score matmuls for the two heads of an e-tile are issued back-to-back with
  explicit tile_position (0,0)/(64,0): K=64 row-tiled matmuls in distinct
  row groups execute concurrently in the PE array, and their outputs land in
  different PSUM banks of one shared [128,1024] tile (one exp per tile).
- Inputs arrive s-block-major ([128, sb, dt, 512]) so each 512-column
  projection group depends on a single 512KB DMA; attention starts as soon
  as the first blocks land instead of after the full input load.
- Softmax normalization is fully distributed: per (head, 512q) slice, a
  single-pass reciprocal_approx_fast on the denominator row feeds a gpsimd
  partition_broadcast, then one DVE multiply writes normalized O^T. No
  cross-head gather, no batched reciprocal, no selector matmuls: keeps the
  PE streaming gap-free (HAM re-throttles the PE clock to 1.2GHz after idle
  windows, so PE gaps cost double).
"""

import re

import numpy as np
import ml_dtypes

import concourse.bass as bass
import concourse.mybir as mybir
from concourse.bass_utils import run_bass_kernel_spmd
from concourse.tile import ScopedClock, TileContext, VectorClock

BF16 = mybir.dt.bfloat16
F32 = mybir.dt.float32
NP_BF16 = ml_dtypes.bfloat16

B, S, D, H, DK = 4, 2048, 512, 8, 64
SCALE = float(1.0 / (np.float32(np.sqrt(DK)) + 1e-8))
E = 256          # head dims per core (4 heads)
NCORES = 8
KT = S // 128    # 16 key tiles of 128
QB = 2           # q blocks of 1024
SB = S // 512    # 4 s-blocks of 512


# ---------------------------------------------------------------------------
# walrus in this container rejects >1 sync-wait command per instruction;
# split the Tile tail drain and hoist excess mid-kernel waits onto NoOps.
# ---------------------------------------------------------------------------

def _clock_entries(vc):
    nums = [int(s) for s in re.findall(r"-?\d+", repr(vc))]
    return [(i, n) for i, n in enumerate(nums) if n > 0]


class SplitDrainTileContext(TileContext):
    def _drain_and_barrier(self, tick_clock, wait_clock):
        nc = self.nc
        for proc, tick in _clock_entries(tick_clock.global_clock):
            vc = VectorClock()
            vc.require_at_least(proc, tick)
            carrier = nc.sync.nop()
            wait_clock.add_sem_waits(carrier.ins, ScopedClock({None: vc}))
        nc.sync.drain()
        nc.all_engine_barrier()
        assert self.sems is not None
        popped = nc._tile_sem_poison_stack.pop()
        assert popped is self._sem_poison
        nc.clear_and_free_semaphores(list(self.sems.allocated().values()))
        nc.all_engine_barrier()


def sanitize_waits(nc, max_waits: int = 1):
    n_split = 0
    for fn in nc.m.functions:
        for bb in fn.blocks:
            new_insts = []
            for inst in bb.instructions:
                si = inst.sync_info
                waits = list(si.on_wait) if si and si.on_wait else []
                if len(waits) > max_waits:
                    keep = waits[-max_waits:]
                    excess = waits[:-max_waits]
                    for i in range(0, len(excess), max_waits):
                        nop = mybir.InstNoOp(
                            name=nc.get_next_instruction_name(), ins=[], outs=[]
                        )
                        nop.engine = inst.engine
                        nop.sync_info = mybir.SyncInfo(
                            on_wait=excess[i : i + max_waits], on_update=[]
                        )
                        new_insts.append(nop)
                    inst.sync_info = mybir.SyncInfo(
                        on_wait=keep, on_update=si.on_update
                    )
                    n_split += 1
                new_insts.append(inst)
            bb.instructions[:] = new_insts
    return n_split


# ---------------------------------------------------------------------------
# kernel builder (one SPMD program; per-core data differs only in in_maps)
# ---------------------------------------------------------------------------

def build_nc(sanitize=True):
    nc = bass.Bass("TRN2", target_bir_lowering=False, debug=False,
                   num_devices=NCORES)

    # x^T tensors arrive host-permuted as [128, sb, dt, 512]: partition p,
    # block (sb, dt) holds row dt*128+p, columns sb*512... of x^T. One DMA
    # per s-block moves 4KB contiguous per partition (near line rate) and is
    # the single dependency for that block's projection group.
    xqT = nc.declare_dram_parameter("xqT", [128, SB, 4, 512], BF16, isOutput=False)
    xkT = nc.declare_dram_parameter("xkT", [128, SB, 4, 512], BF16, isOutput=False)
    xvT = nc.declare_dram_parameter("xvT", [128, SB, 4, 512], BF16, isOutput=False)
    wqT = nc.declare_dram_parameter("wqT", [128, 4, E], BF16, isOutput=False)
    wkT = nc.declare_dram_parameter("wkT", [128, 4, E], BF16, isOutput=False)
    wvT = nc.declare_dram_parameter("wvT", [128, 4, E], BF16, isOutput=False)
    woT = nc.declare_dram_parameter("woT", [E, D], BF16, isOutput=False)
    bqs = nc.declare_dram_parameter("bqs", [E], F32, isOutput=False)
    bks = nc.declare_dram_parameter("bks", [E], F32, isOutput=False)
    bvb = nc.declare_dram_parameter("bvb", [128, E], F32, isOutput=False)
    e4d = nc.declare_dram_parameter("e4d", [4, 256], F32, isOutput=False)
    yT = nc.declare_dram_parameter("yT", [D, S], F32, isOutput=True)

    Exp = mybir.ActivationFunctionType.Exp

    with SplitDrainTileContext(nc) as tc:
        with tc.sbuf_pool(name="persist", bufs=1) as P:
            QT = P.tile([128, 2, S], BF16)    # e-tiles x queries
            KTt = P.tile([128, 2, S], BF16)
            VA = P.tile([128, KT, 4 * 65], BF16)  # [V_h | ones] per head
            OT = P.tile([128, 2, S], BF16)
            WOT = P.tile([128, 2, D], BF16)
            BQ = P.tile([128, 2], F32)
            BK = P.tile([128, 2], F32)
            BVB = P.tile([128, E], F32)
            # E4[k, j*64+m] = (k==j): selector that broadcasts row j of a
            # [4, 512] tile across 64 partitions via a K=4 matmul.
            E4 = P.tile([4, 256], mybir.dt.float32r)
            E4F = P.tile([4, 256], F32)
            XQT = P.tile([128, SB, 4, 512], BF16)
            XKT = P.tile([128, SB, 4, 512], BF16)
            XVT = P.tile([128, SB, 4, 512], BF16)
            WQ = P.tile([128, 4, E], BF16)
            WK = P.tile([128, 4, E], BF16)
            WVs = P.tile([128, 4, E], BF16)

            # softmax-denominator ones columns of V_aug
            for kt in range(KT):
                va_h = VA[:, kt, :].rearrange("p (h c) -> p h c", c=65)
                nc.vector.memset(va_h[:, :, 64:65], 1.0)
            nc.scalar.dma_start(out=E4F[:, :], in_=e4d[:, :])
            with nc.allow_low_precision(reason="exact 0/1 rounded to fp32r"):
                nc.vector.tensor_copy(E4[:, :], E4F[:, :])

            # ---- input DMAs spread over the three DMA-capable queues: X_K
            # on sync, X_Q on scalar, weights/biases then X_V on gpsimd, so
            # the first projection's inputs land in parallel.
            nc.gpsimd.dma_start(out=WK[:, :, :], in_=wkT[:, :, :])
            nc.gpsimd.dma_start(out=WQ[:, :, :], in_=wqT[:, :, :])
            nc.gpsimd.dma_start(
                out=BQ[:, :], in_=bqs[:].rearrange("(c p) -> p c", p=128)
            )
            nc.gpsimd.dma_start(
                out=BK[:, :], in_=bks[:].rearrange("(c p) -> p c", p=128)
            )
            nc.gpsimd.dma_start(out=WVs[:, :, :], in_=wvT[:, :, :])
            nc.gpsimd.dma_start(out=BVB[:, :], in_=bvb[:, :])
            for sb in range(SB):
                nc.sync.dma_start(out=XKT[:, sb, :, :], in_=xkT[:, sb, :, :])
                nc.scalar.dma_start(out=XQT[:, sb, :, :], in_=xqT[:, sb, :, :])
                nc.gpsimd.dma_start(out=XVT[:, sb, :, :], in_=xvT[:, sb, :, :])
            for et in range(2):
                sl = slice(et * 128, (et + 1) * 128)
                nc.gpsimd.dma_start(out=WOT[:, et, :], in_=woT[sl, :])

            # one shared ring of [128, 512] fp32 PSUM tiles (2 banks) serves
            # the projections, attn@V accumulators, the denominator
            # broadcast, and the output projection; scores get the other 6.
            with tc.psum_pool(name="pp", bufs=2) as PP:
                # Q^T / K^T projections, ordered so the earliest score
                # matmuls (et0, low kt / low q) unblock first.
                def proj_qk(xt, wt, out, bias, et, sb):
                    ssl = slice(sb * 512, (sb + 1) * 512)
                    ps = PP.tile([128, 512], F32, tag="ps512")
                    for dt in range(4):
                        nc.tensor.matmul(
                            ps[:, :],
                            lhsT=wt[:, dt, et * 128:(et + 1) * 128],
                            rhs=xt[:, sb, dt, :],
                            start=(dt == 0),
                            stop=(dt == 3),
                        )
                    nc.vector.tensor_scalar_add(
                        out[:, et, ssl], ps[:, :], bias[:, et:et + 1]
                    )

                order = [
                    (0, 0, "k"), (0, 0, "q"), (0, 1, "q"), (0, 1, "k"),
                    (0, 2, "k"), (0, 3, "k"),
                    (1, 0, "k"), (1, 0, "q"), (1, 1, "q"), (1, 1, "k"),
                    (1, 2, "k"), (1, 3, "k"),
                    (0, 2, "q"), (0, 3, "q"), (1, 2, "q"), (1, 3, "q"),
                ]
                for et, sb, which in order:
                    if which == "k":
                        proj_qk(XKT, WK, KTt, BK, et, sb)
                    else:
                        proj_qk(XQT, WQ, QT, BQ, et, sb)

                # V: natural [s, e] + bias, interleaved [V_h | ones]
                for kt in range(KT):
                    psv = PP.tile([128, 512], F32, tag="ps512")
                    sb, off = divmod(kt * 128, 512)
                    for dt in range(4):
                        nc.tensor.matmul(
                            psv[:, 0:E],
                            lhsT=XVT[:, sb, dt, off:off + 128],
                            rhs=WVs[:, dt, :],
                            start=(dt == 0),
                            stop=(dt == 3),
                        )
                    # one strided DVE add writes all four 64-col V blocks
                    va_v = VA[:, kt, :].rearrange("p (h c) -> p h c", c=65)
                    psv_h = psv[:, 0:E].rearrange("p (h c) -> p h c", c=64)
                    bvb_h = BVB[:, :].rearrange("p (h c) -> p h c", c=64)
                    nc.vector.tensor_add(
                        va_v[:, :, 0:64], psv_h[:, :, :], bvb_h[:, :, :]
                    )

                # ---- attention: per (qb, hp): scores+exp stream per
                # (kt, q-half), then attn@V + distributed normalization.
                with tc.sbuf_pool(name="ptp", bufs=36) as PTP, \
                     tc.sbuf_pool(name="nrm", bufs=4) as NRM, \
                     tc.sbuf_pool(name="yo", bufs=2) as YO, \
                     tc.psum_pool(name="scp", bufs=3) as SCP:
                    for qb in range(QB):
                        q0 = qb * 1024
                        for hp in range(2):
                            et = hp
                            pts = {}
                            with tc.high_priority(offset=300):
                                for kt in range(KT):
                                    ksl = slice(kt * 128, (kt + 1) * 128)
                                    for qh in range(2):
                                        qsl = slice(q0 + qh * 512,
                                                    q0 + qh * 512 + 512)
                                        sc = SCP.tile([128, 1024], F32,
                                                      tag="sc")
                                        # two K=64 heads in distinct PE row
                                        # groups -> concurrent; outputs in
                                        # the tile's two PSUM banks.
                                        for hh in range(2):
                                            hsl = slice(hh * 64, hh * 64 + 64)
                                            nc.tensor.matmul(
                                                sc[:, hh * 512:(hh + 1) * 512],
                                                lhsT=KTt[hsl, et, ksl],
                                                rhs=QT[hsl, et, qsl],
                                                start=True,
                                                stop=True,
                                                tile_position=(hh * 64, 0),
                                            )
                                        pt = PTP.tile([128, 1024], BF16,
                                                      tag="pt")
                                        nc.scalar.activation(
                                            pt[:, :], sc[:, :], Exp,
                                            scale=SCALE,
                                        )
                                        pts[qh, kt] = pt
                            # attn@V, row-split into two concurrent K=64
                            # matmul streams (PE row groups 0/64) writing
                            # separate PSUM accumulators; DVE merges the
                            # halves. Denominator rows DMA into the per-hp
                            # sums tile so the single [4,512] reciprocal
                            # runs per head-pair (hp0's normalization
                            # overlaps hp1's attention).
                            sums = NRM.tile([4, 512], F32, tag="sums",
                                            bufs=2)
                            ous = {}
                            for hh in range(2):
                                h = hp * 2 + hh
                                for sq in range(2):
                                    j2 = hh * 2 + sq
                                    opsA = PP.tile([128, 512], F32,
                                                   tag="ps512")
                                    opsB = PP.tile([128, 512], F32,
                                                   tag="ps512")
                                    for kt in range(KT):
                                        for lo, op_t in ((0, opsA),
                                                         (64, opsB)):
                                            nc.tensor.matmul(
                                                op_t[0:65, :],
                                                lhsT=VA[lo:lo + 64, kt,
                                                        h * 65:(h + 1) * 65],
                                                rhs=pts[sq, kt][lo:lo + 64,
                                                                hh * 512:
                                                                (hh + 1) * 512],
                                                start=(kt == 0),
                                                stop=(kt == KT - 1),
                                                tile_position=(lo, 0),
                                            )
                                    oa = NRM.tile([65, 512], F32, tag="oa",
                                                  bufs=3)
                                    nc.vector.tensor_copy(oa[:, :],
                                                          opsA[0:65, :])
                                    ou = NRM.tile([65, 512], F32, tag="ou",
                                                  bufs=5)
                                    nc.vector.tensor_add(ou[:, :], oa[:, :],
                                                         opsB[0:65, :])
                                    nc.sync.dma_start(
                                        out=sums[j2:j2 + 1, :],
                                        in_=ou[64:65, :],
                                    )
                                    ous[j2] = ou
                            rcb = NRM.tile([4, 512], mybir.dt.float32r,
                                           tag="rcb", bufs=2)
                            with nc.allow_low_precision(
                                reason="softmax 1/denom rounded to fp32r "
                                "for the selector-matmul broadcast"
                            ):
                                nc.vector.reciprocal(rcb[:, :], sums[:, :])
                            for hh in range(2):
                                hsl = slice(hh * 64, hh * 64 + 64)
                                for sq in range(2):
                                    j2 = hh * 2 + sq
                                    s0 = q0 + sq * 512
                                    ssl = slice(s0, s0 + 512)
                                    bc = PP.tile([128, 512], F32,
                                                 tag="ps512")
                                    nc.tensor.matmul(
                                        bc[0:64, :],
                                        lhsT=E4[:, j2 * 64:(j2 + 1) * 64],
                                        rhs=rcb[:, :],
                                        start=True,
                                        stop=True,
                                    )
                                    nc.vector.tensor_mul(
                                        OT[hsl, hp, ssl], ous[j2][0:64, :],
                                        bc[0:64, :]
                                    )
                        # output projection, per 512-q slice so the first
                        # slice overlaps the second slice's normalization
                        for sq in range(2):
                            s0 = q0 + sq * 512
                            ssl = slice(s0, s0 + 512)
                            for fc in range(4):
                                yp = PP.tile([128, 512], F32, tag="ps512")
                                for et in range(2):
                                    nc.tensor.matmul(
                                        yp[:, :],
                                        lhsT=WOT[:, et, fc * 128:(fc + 1) * 128],
                                        rhs=OT[:, et, ssl],
                                        start=(et == 0),
                                        stop=(et == 1),
                                    )
                                ys = YO.tile([128, 512], F32, tag="ys")
                                nc.vector.tensor_copy(ys[:, :], yp[:, :])
                                nc.sync.dma_start(
                                    out=yT[fc * 128:(fc + 1) * 128, ssl],
                                    in_=ys[:, :],
                                )

    if sanitize:
        sanitize_waits(nc)
    return nc


def _perm_xt(x):
    # (S, D) -> x^T laid out [128, sb, dt, 512]: partition p, block (sb, dt)
    # = row dt*128+p of x^T, columns sb*512:(sb+1)*512
    xt = x.T.astype(NP_BF16)                      # (512, S)
    return np.ascontiguousarray(
        xt.reshape(4, 128, SB, 512).transpose(1, 2, 0, 3)
    )


def _perm_w(w):
    # (E, D) slice of torch weight -> W^T laid out [128, dt, E]
    wt = w.T.astype(NP_BF16)                      # (D, E)
    return np.ascontiguousarray(wt.reshape(4, 128, E).transpose(1, 0, 2))


def _e4():
    e = np.zeros((4, 256), dtype=np.float32)
    for j in range(4):
        e[j, j * 64:(j + 1) * 64] = 1.0
    return e


def make_in_maps(query, key, value, Wq, bq, Wk, bk, Wv, bv, Wo, bo):
    in_maps = []
    for c in range(NCORES):
        b, g = divmod(c, 2)
        eo = g * E
        esl = slice(eo, eo + E)
        in_maps.append({
            "xqT": _perm_xt(query[b]),
            "xkT": _perm_xt(key[b]),
            "xvT": _perm_xt(value[b]),
            "wqT": _perm_w(Wq[esl, :]),
            "wkT": _perm_w(Wk[esl, :]),
            "wvT": _perm_w(Wv[esl, :]),
            "woT": Wo[:, esl].T.astype(NP_BF16),
            "bqs": np.ascontiguousarray(bq[esl], dtype=np.float32),
            "bks": np.ascontiguousarray(bk[esl], dtype=np.float32),
            "bvb": np.ascontiguousarray(
                np.broadcast_to(bv[esl], (128, E)), dtype=np.float32
            ),
            "e4d": _e4(),
        })
    return in_maps


def gather(results, bo):
    out = np.empty((B, S, D), dtype=np.float32)
    for b in range(B):
        yt = results[2 * b]["yT"] + results[2 * b + 1]["yT"]
        out[b] = yt.T + np.asarray(bo, dtype=np.float32)
    return out


_NC = None


def kernel(query, key, value, Wq, bq, Wk, bk, Wv, bv, Wo, bo, **run_kwargs):
    global _NC
    if _NC is None:
        _NC = build_nc()
    args = [np.asarray(a) for a in
            (query, key, value, Wq, bq, Wk, bk, Wv, bv, Wo, bo)]
    in_maps = make_in_maps(*args)
    res = run_bass_kernel_spmd(_NC, in_maps, list(range(NCORES)), **run_kwargs)
    out = gather(res.results, args[10])
    if run_kwargs:
        return out, res
    return out


# revision 29
# speedup vs baseline: 1.3240x; 1.3240x over previous
"""Multi-head attention (B=4, S=2048, D=512, H=8) on 8 trn2 NeuronCores.

Sharding: core c handles batch b=c//2, head-group g=c%2 (4 heads, 256 of the
512 projection dims). Each core runs the full fused pipeline for its four
heads - QKV projection, scores^T = K_h Q_h^T, exp (softmax numerator),
attn @ V with a folded ones-column producing the softmax denominators,
normalization, and its partial output projection y^T = Wo_slice^T.T @ O^T.
The host sums the two partial y^T per batch and adds the output bias.

Key performance structure (vs the v1 kernel):
- Score matmuls for the two heads of an e-tile are issued back-to-back with
  explicit tile_position (0,0)/(64,0): K=64 row-tiled matmuls in distinct
  row groups execute concurrently in the PE array, and their outputs land in
  different PSUM banks of one shared [128,1024] tile (one exp per tile).
- Inputs arrive s-block-major ([128, sb, dt, 512]) so each 512-column
  projection group depends on a single 512KB DMA; attention starts as soon
  as the first blocks land instead of after the full input load.
- Softmax normalization is fully distributed: per (head, 512q) slice, a
  single-pass reciprocal_approx_fast on the denominator row feeds a gpsimd
  partition_broadcast, then one DVE multiply writes normalized O^T. No
  cross-head gather, no batched reciprocal, no selector matmuls: keeps the
  PE streaming gap-free (HAM re-throttles the PE clock to 1.2GHz after idle
  windows, so PE gaps cost double).
"""

import re

import numpy as np
import ml_dtypes

import concourse.bass as bass
import concourse.mybir as mybir
from concourse.bass_utils import run_bass_kernel_spmd
from concourse.tile import ScopedClock, TileContext, VectorClock

BF16 = mybir.dt.bfloat16
F32 = mybir.dt.float32
NP_BF16 = ml_dtypes.bfloat16

B, S, D, H, DK = 4, 2048, 512, 8, 64
SCALE = float(1.0 / (np.float32(np.sqrt(DK)) + 1e-8))
E = 256          # head dims per core (4 heads)
NCORES = 8
KT = S // 128    # 16 key tiles of 128
QB = 2           # q blocks of 1024
SB = S // 512    # 4 s-blocks of 512


# ---------------------------------------------------------------------------
# walrus in this container rejects >1 sync-wait command per instruction;
# split the Tile tail drain and hoist excess mid-kernel waits onto NoOps.
# ---------------------------------------------------------------------------

def _clock_entries(vc):
    nums = [int(s) for s in re.findall(r"-?\d+", repr(vc))]
    return [(i, n) for i, n in enumerate(nums) if n > 0]


class SplitDrainTileContext(TileContext):
    def _drain_and_barrier(self, tick_clock, wait_clock):
        nc = self.nc
        for proc, tick in _clock_entries(tick_clock.global_clock):
            vc = VectorClock()
            vc.require_at_least(proc, tick)
            carrier = nc.sync.nop()
            wait_clock.add_sem_waits(carrier.ins, ScopedClock({None: vc}))
        nc.sync.drain()
        nc.all_engine_barrier()
        assert self.sems is not None
        popped = nc._tile_sem_poison_stack.pop()
        assert popped is self._sem_poison
        nc.clear_and_free_semaphores(list(self.sems.allocated().values()))
        nc.all_engine_barrier()


def sanitize_waits(nc, max_waits: int = 1):
    n_split = 0
    for fn in nc.m.functions:
        for bb in fn.blocks:
            new_insts = []
            for inst in bb.instructions:
                si = inst.sync_info
                waits = list(si.on_wait) if si and si.on_wait else []
                if len(waits) > max_waits:
                    keep = waits[-max_waits:]
                    excess = waits[:-max_waits]
                    for i in range(0, len(excess), max_waits):
                        nop = mybir.InstNoOp(
                            name=nc.get_next_instruction_name(), ins=[], outs=[]
                        )
                        nop.engine = inst.engine
                        nop.sync_info = mybir.SyncInfo(
                            on_wait=excess[i : i + max_waits], on_update=[]
                        )
                        new_insts.append(nop)
                    inst.sync_info = mybir.SyncInfo(
                        on_wait=keep, on_update=si.on_update
                    )
                    n_split += 1
                new_insts.append(inst)
            bb.instructions[:] = new_insts
    return n_split


# ---------------------------------------------------------------------------
# kernel builder (one SPMD program; per-core data differs only in in_maps)
# ---------------------------------------------------------------------------

def build_nc(sanitize=True):
    nc = bass.Bass("TRN2", target_bir_lowering=False, debug=False,
                   num_devices=NCORES)

    # x^T tensors arrive host-permuted as [128, sb, dt, 512]: partition p,
    # block (sb, dt) holds row dt*128+p, columns sb*512... of x^T. One DMA
    # per s-block moves 4KB contiguous per partition (near line rate) and is
    # the single dependency for that block's projection group.
    xqT = nc.declare_dram_parameter("xqT", [128, SB, 4, 512], BF16, isOutput=False)
    xkT = nc.declare_dram_parameter("xkT", [128, SB, 4, 512], BF16, isOutput=False)
    xvT = nc.declare_dram_parameter("xvT", [128, SB, 4, 512], BF16, isOutput=False)
    wqT = nc.declare_dram_parameter("wqT", [128, 4, E], BF16, isOutput=False)
    wkT = nc.declare_dram_parameter("wkT", [128, 4, E], BF16, isOutput=False)
    wvT = nc.declare_dram_parameter("wvT", [128, 4, E], BF16, isOutput=False)
    woT = nc.declare_dram_parameter("woT", [E, D], BF16, isOutput=False)
    bqs = nc.declare_dram_parameter("bqs", [E], F32, isOutput=False)
    bks = nc.declare_dram_parameter("bks", [E], F32, isOutput=False)
    bvb = nc.declare_dram_parameter("bvb", [128, E], F32, isOutput=False)
    e4d = nc.declare_dram_parameter("e4d", [4, 256], F32, isOutput=False)
    yT = nc.declare_dram_parameter("yT", [D, S], F32, isOutput=True)

    Exp = mybir.ActivationFunctionType.Exp

    with SplitDrainTileContext(nc) as tc:
        with tc.sbuf_pool(name="persist", bufs=1) as P:
            QT = P.tile([128, 2, S], BF16)    # e-tiles x queries
            KTt = P.tile([128, 2, S], BF16)
            VA = P.tile([128, KT, 4 * 65], BF16)  # [V_h | ones] per head
            OT = P.tile([128, 2, S], BF16)
            WOT = P.tile([128, 2, D], BF16)
            BQ = P.tile([128, 2], F32)
            BK = P.tile([128, 2], F32)
            BVB = P.tile([128, E], F32)
            # E4[k, j*64+m] = (k==j): selector that broadcasts row j of a
            # [4, 512] tile across 64 partitions via a K=4 matmul.
            E4 = P.tile([4, 256], mybir.dt.float32r)
            E4F = P.tile([4, 256], F32)
            XQT = P.tile([128, SB, 4, 512], BF16)
            XKT = P.tile([128, SB, 4, 512], BF16)
            XVT = P.tile([128, SB, 4, 512], BF16)
            WQ = P.tile([128, 4, E], BF16)
            WK = P.tile([128, 4, E], BF16)
            WVs = P.tile([128, 4, E], BF16)

            # softmax-denominator ones columns of V_aug
            for kt in range(KT):
                va_h = VA[:, kt, :].rearrange("p (h c) -> p h c", c=65)
                nc.vector.memset(va_h[:, :, 64:65], 1.0)
            nc.scalar.dma_start(out=E4F[:, :], in_=e4d[:, :])
            with nc.allow_low_precision(reason="exact 0/1 rounded to fp32r"):
                nc.vector.tensor_copy(E4[:, :], E4F[:, :])

            # ---- input DMAs spread over the three DMA-capable queues: X_K
            # on sync, X_Q on scalar, weights/biases then X_V on gpsimd, so
            # the first projection's inputs land in parallel.
            nc.gpsimd.dma_start(out=WK[:, :, :], in_=wkT[:, :, :])
            nc.gpsimd.dma_start(out=WQ[:, :, :], in_=wqT[:, :, :])
            nc.gpsimd.dma_start(
                out=BQ[:, :], in_=bqs[:].rearrange("(c p) -> p c", p=128)
            )
            nc.gpsimd.dma_start(
                out=BK[:, :], in_=bks[:].rearrange("(c p) -> p c", p=128)
            )
            nc.gpsimd.dma_start(out=WVs[:, :, :], in_=wvT[:, :, :])
            nc.gpsimd.dma_start(out=BVB[:, :], in_=bvb[:, :])
            for sb in range(SB):
                nc.sync.dma_start(out=XKT[:, sb, :, :], in_=xkT[:, sb, :, :])
                nc.scalar.dma_start(out=XQT[:, sb, :, :], in_=xqT[:, sb, :, :])
                nc.gpsimd.dma_start(out=XVT[:, sb, :, :], in_=xvT[:, sb, :, :])
            for et in range(2):
                sl = slice(et * 128, (et + 1) * 128)
                nc.gpsimd.dma_start(out=WOT[:, et, :], in_=woT[sl, :])

            # one shared ring of [128, 512] fp32 PSUM tiles (2 banks) serves
            # the projections, attn@V accumulators, the denominator
            # broadcast, and the output projection; scores get the other 6.
            with tc.psum_pool(name="pp", bufs=2) as PP:
                # Q^T / K^T projections, ordered so the earliest score
                # matmuls (et0, low kt / low q) unblock first.
                def proj_qk(xt, wt, out, bias, et, sb):
                    ssl = slice(sb * 512, (sb + 1) * 512)
                    ps = PP.tile([128, 512], F32, tag="ps512")
                    for dt in range(4):
                        nc.tensor.matmul(
                            ps[:, :],
                            lhsT=wt[:, dt, et * 128:(et + 1) * 128],
                            rhs=xt[:, sb, dt, :],
                            start=(dt == 0),
                            stop=(dt == 3),
                        )
                    nc.vector.tensor_scalar_add(
                        out[:, et, ssl], ps[:, :], bias[:, et:et + 1]
                    )

                order = [
                    (0, 0, "k"), (0, 0, "q"), (0, 1, "q"), (0, 1, "k"),
                    (0, 2, "k"), (0, 3, "k"),
                    (1, 0, "k"), (1, 0, "q"), (1, 1, "q"), (1, 1, "k"),
                    (1, 2, "k"), (1, 3, "k"),
                    (0, 2, "q"), (0, 3, "q"), (1, 2, "q"), (1, 3, "q"),
                ]
                for et, sb, which in order:
                    if which == "k":
                        proj_qk(XKT, WK, KTt, BK, et, sb)
                    else:
                        proj_qk(XQT, WQ, QT, BQ, et, sb)

                # V: natural [s, e] + bias, interleaved [V_h | ones]
                for kt in range(KT):
                    psv = PP.tile([128, 512], F32, tag="ps512")
                    sb, off = divmod(kt * 128, 512)
                    for dt in range(4):
                        nc.tensor.matmul(
                            psv[:, 0:E],
                            lhsT=XVT[:, sb, dt, off:off + 128],
                            rhs=WVs[:, dt, :],
                            start=(dt == 0),
                            stop=(dt == 3),
                        )
                    # one strided DVE add writes all four 64-col V blocks
                    va_v = VA[:, kt, :].rearrange("p (h c) -> p h c", c=65)
                    psv_h = psv[:, 0:E].rearrange("p (h c) -> p h c", c=64)
                    bvb_h = BVB[:, :].rearrange("p (h c) -> p h c", c=64)
                    nc.vector.tensor_add(
                        va_v[:, :, 0:64], psv_h[:, :, :], bvb_h[:, :, :]
                    )

                # ---- attention: per (qb, hp): scores+exp stream per
                # (kt, q-half), then attn@V + distributed normalization.
                with tc.sbuf_pool(name="ptp", bufs=36) as PTP, \
                     tc.sbuf_pool(name="nrm", bufs=4) as NRM, \
                     tc.sbuf_pool(name="yo", bufs=2) as YO, \
                     tc.psum_pool(name="scp", bufs=3) as SCP:
                    for qb in range(QB):
                        q0 = qb * 1024
                        for hp in range(2):
                            et = hp
                            pts = {}
                            with tc.high_priority(offset=300):
                                for kt in range(KT):
                                    ksl = slice(kt * 128, (kt + 1) * 128)
                                    for qh in range(2):
                                        qsl = slice(q0 + qh * 512,
                                                    q0 + qh * 512 + 512)
                                        sc = SCP.tile([128, 1024], F32,
                                                      tag="sc")
                                        # two K=64 heads in distinct PE row
                                        # groups -> concurrent; outputs in
                                        # the tile's two PSUM banks.
                                        for hh in range(2):
                                            hsl = slice(hh * 64, hh * 64 + 64)
                                            nc.tensor.matmul(
                                                sc[:, hh * 512:(hh + 1) * 512],
                                                lhsT=KTt[hsl, et, ksl],
                                                rhs=QT[hsl, et, qsl],
                                                start=True,
                                                stop=True,
                                                tile_position=(hh * 64, 0),
                                            )
                                        pt = PTP.tile([128, 1024], BF16,
                                                      tag="pt")
                                        nc.scalar.activation(
                                            pt[:, :], sc[:, :], Exp,
                                            scale=SCALE,
                                        )
                                        pts[qh, kt] = pt
                            # attn@V; denominator rows (the VA ones-column
                            # output) stage through SBUF then DMA into the
                            # per-hp sums tile so the single [4,512]
                            # reciprocal runs per head-pair (hp0's
                            # normalization overlaps hp1's attention).
                            sums = NRM.tile([4, 512], F32, tag="sums",
                                            bufs=2)
                            ous = {}
                            for hh in range(2):
                                h = hp * 2 + hh
                                for sq in range(2):
                                    j2 = hh * 2 + sq
                                    ops = PP.tile([128, 512], F32,
                                                  tag="ps512")
                                    for kt in range(KT):
                                        nc.tensor.matmul(
                                            ops[0:65, :],
                                            lhsT=VA[:, kt, h * 65:(h + 1) * 65],
                                            rhs=pts[sq, kt][:,
                                                            hh * 512:
                                                            (hh + 1) * 512],
                                            start=(kt == 0),
                                            stop=(kt == KT - 1),
                                        )
                                    ou = NRM.tile([65, 512], F32, tag="ou",
                                                  bufs=5)
                                    nc.vector.tensor_copy(ou[:, :],
                                                          ops[0:65, :])
                                    nc.sync.dma_start(
                                        out=sums[j2:j2 + 1, :],
                                        in_=ou[64:65, :],
                                    )
                                    ous[j2] = ou
                            rcb = NRM.tile([4, 512], mybir.dt.float32r,
                                           tag="rcb", bufs=2)
                            with nc.allow_low_precision(
                                reason="softmax 1/denom rounded to fp32r "
                                "for the selector-matmul broadcast"
                            ):
                                nc.vector.reciprocal(rcb[:, :], sums[:, :])
                            for hh in range(2):
                                hsl = slice(hh * 64, hh * 64 + 64)
                                for sq in range(2):
                                    j2 = hh * 2 + sq
                                    s0 = q0 + sq * 512
                                    ssl = slice(s0, s0 + 512)
                                    bc = PP.tile([128, 512], F32,
                                                 tag="ps512")
                                    nc.tensor.matmul(
                                        bc[0:64, :],
                                        lhsT=E4[:, j2 * 64:(j2 + 1) * 64],
                                        rhs=rcb[:, :],
                                        start=True,
                                        stop=True,
                                    )
                                    nc.vector.tensor_mul(
                                        OT[hsl, hp, ssl], ous[j2][0:64, :],
                                        bc[0:64, :]
                                    )
                        # output projection, per 512-q slice so the first
                        # slice overlaps the second slice's normalization
                        for sq in range(2):
                            s0 = q0 + sq * 512
                            ssl = slice(s0, s0 + 512)
                            for fc in range(4):
                                yp = PP.tile([128, 512], F32, tag="ps512")
                                for et in range(2):
                                    nc.tensor.matmul(
                                        yp[:, :],
                                        lhsT=WOT[:, et, fc * 128:(fc + 1) * 128],
                                        rhs=OT[:, et, ssl],
                                        start=(et == 0),
                                        stop=(et == 1),
                                    )
                                ys = YO.tile([128, 512], F32, tag="ys")
                                nc.vector.tensor_copy(ys[:, :], yp[:, :])
                                nc.sync.dma_start(
                                    out=yT[fc * 128:(fc + 1) * 128, ssl],
                                    in_=ys[:, :],
                                )

    if sanitize:
        sanitize_waits(nc)
    return nc


def _perm_xt(x):
    # (S, D) -> x^T laid out [128, sb, dt, 512]: partition p, block (sb, dt)
    # = row dt*128+p of x^T, columns sb*512:(sb+1)*512
    xt = x.T.astype(NP_BF16)                      # (512, S)
    return np.ascontiguousarray(
        xt.reshape(4, 128, SB, 512).transpose(1, 2, 0, 3)
    )


def _perm_w(w):
    # (E, D) slice of torch weight -> W^T laid out [128, dt, E]
    wt = w.T.astype(NP_BF16)                      # (D, E)
    return np.ascontiguousarray(wt.reshape(4, 128, E).transpose(1, 0, 2))


def _e4():
    e = np.zeros((4, 256), dtype=np.float32)
    for j in range(4):
        e[j, j * 64:(j + 1) * 64] = 1.0
    return e


def make_in_maps(query, key, value, Wq, bq, Wk, bk, Wv, bv, Wo, bo):
    in_maps = []
    for c in range(NCORES):
        b, g = divmod(c, 2)
        eo = g * E
        esl = slice(eo, eo + E)
        in_maps.append({
            "xqT": _perm_xt(query[b]),
            "xkT": _perm_xt(key[b]),
            "xvT": _perm_xt(value[b]),
            "wqT": _perm_w(Wq[esl, :]),
            "wkT": _perm_w(Wk[esl, :]),
            "wvT": _perm_w(Wv[esl, :]),
            "woT": Wo[:, esl].T.astype(NP_BF16),
            "bqs": np.ascontiguousarray(bq[esl], dtype=np.float32),
            "bks": np.ascontiguousarray(bk[esl], dtype=np.float32),
            "bvb": np.ascontiguousarray(
                np.broadcast_to(bv[esl], (128, E)), dtype=np.float32
            ),
            "e4d": _e4(),
        })
    return in_maps


def gather(results, bo):
    out = np.empty((B, S, D), dtype=np.float32)
    for b in range(B):
        yt = results[2 * b]["yT"] + results[2 * b + 1]["yT"]
        out[b] = yt.T + np.asarray(bo, dtype=np.float32)
    return out


_NC = None


def kernel(query, key, value, Wq, bq, Wk, bk, Wv, bv, Wo, bo, **run_kwargs):
    global _NC
    if _NC is None:
        _NC = build_nc()
    args = [np.asarray(a) for a in
            (query, key, value, Wq, bq, Wk, bk, Wv, bv, Wo, bo)]
    in_maps = make_in_maps(*args)
    res = run_bass_kernel_spmd(_NC, in_maps, list(range(NCORES)), **run_kwargs)
    out = gather(res.results, args[10])
    if run_kwargs:
        return out, res
    return out


# revision 30
# speedup vs baseline: 1.3420x; 1.0136x over previous
"""Multi-head attention (B=4, S=2048, D=512, H=8) on 8 trn2 NeuronCores.

Sharding: core c handles batch b=c//2, head-group g=c%2 (4 heads, 256 of the
512 projection dims). Each core runs the full fused pipeline for its four
heads - QKV projection, scores^T = K_h Q_h^T, exp (softmax numerator),
attn @ V with a folded ones-column producing the softmax denominators,
normalization, and its partial output projection y^T = Wo_slice^T.T @ O^T.
The host sums the two partial y^T per batch and adds the output bias.

Key performance structure (vs the v1 kernel):
- Score matmuls for the two heads of an e-tile are issued back-to-back with
  explicit tile_position (0,0)/(64,0): K=64 row-tiled matmuls in distinct
  row groups execute concurrently in the PE array, and their outputs land in
  different PSUM banks of one shared [128,1024] tile (one exp per tile).
- Inputs arrive s-block-major ([128, sb, dt, 512]) so each 512-column
  projection group depends on a single 512KB DMA; attention starts as soon
  as the first blocks land instead of after the full input load.
- Softmax normalization is fully distributed: per (head, 512q) slice, a
  single-pass reciprocal_approx_fast on the denominator row feeds a gpsimd
  partition_broadcast, then one DVE multiply writes normalized O^T. No
  cross-head gather, no batched reciprocal, no selector matmuls: keeps the
  PE streaming gap-free (HAM re-throttles the PE clock to 1.2GHz after idle
  windows, so PE gaps cost double).
"""

import re

import numpy as np
import ml_dtypes

import concourse.bass as bass
import concourse.mybir as mybir
from concourse.bass_utils import run_bass_kernel_spmd
from concourse.tile import ScopedClock, TileContext, VectorClock

BF16 = mybir.dt.bfloat16
F32 = mybir.dt.float32
NP_BF16 = ml_dtypes.bfloat16

B, S, D, H, DK = 4, 2048, 512, 8, 64
SCALE = float(1.0 / (np.float32(np.sqrt(DK)) + 1e-8))
E = 256          # head dims per core (4 heads)
NCORES = 8
KT = S // 128    # 16 key tiles of 128
QB = 2           # q blocks of 1024
SB = S // 512    # 4 s-blocks of 512


# ---------------------------------------------------------------------------
# walrus in this container rejects >1 sync-wait command per instruction;
# split the Tile tail drain and hoist excess mid-kernel waits onto NoOps.
# ---------------------------------------------------------------------------

def _clock_entries(vc):
    nums = [int(s) for s in re.findall(r"-?\d+", repr(vc))]
    return [(i, n) for i, n in enumerate(nums) if n > 0]


class SplitDrainTileContext(TileContext):
    def _drain_and_barrier(self, tick_clock, wait_clock):
        nc = self.nc
        for proc, tick in _clock_entries(tick_clock.global_clock):
            vc = VectorClock()
            vc.require_at_least(proc, tick)
            carrier = nc.sync.nop()
            wait_clock.add_sem_waits(carrier.ins, ScopedClock({None: vc}))
        nc.sync.drain()
        nc.all_engine_barrier()
        assert self.sems is not None
        popped = nc._tile_sem_poison_stack.pop()
        assert popped is self._sem_poison
        nc.clear_and_free_semaphores(list(self.sems.allocated().values()))
        nc.all_engine_barrier()


def sanitize_waits(nc, max_waits: int = 1):
    n_split = 0
    for fn in nc.m.functions:
        for bb in fn.blocks:
            new_insts = []
            for inst in bb.instructions:
                si = inst.sync_info
                waits = list(si.on_wait) if si and si.on_wait else []
                if len(waits) > max_waits:
                    keep = waits[-max_waits:]
                    excess = waits[:-max_waits]
                    for i in range(0, len(excess), max_waits):
                        nop = mybir.InstNoOp(
                            name=nc.get_next_instruction_name(), ins=[], outs=[]
                        )
                        nop.engine = inst.engine
                        nop.sync_info = mybir.SyncInfo(
                            on_wait=excess[i : i + max_waits], on_update=[]
                        )
                        new_insts.append(nop)
                    inst.sync_info = mybir.SyncInfo(
                        on_wait=keep, on_update=si.on_update
                    )
                    n_split += 1
                new_insts.append(inst)
            bb.instructions[:] = new_insts
    return n_split


# ---------------------------------------------------------------------------
# kernel builder (one SPMD program; per-core data differs only in in_maps)
# ---------------------------------------------------------------------------

def build_nc(sanitize=True):
    nc = bass.Bass("TRN2", target_bir_lowering=False, debug=False,
                   num_devices=NCORES)

    # x^T tensors arrive host-permuted as [128, sb, dt, 512]: partition p,
    # block (sb, dt) holds row dt*128+p, columns sb*512... of x^T. One DMA
    # per s-block moves 4KB contiguous per partition (near line rate) and is
    # the single dependency for that block's projection group.
    xqT = nc.declare_dram_parameter("xqT", [128, SB, 4, 512], BF16, isOutput=False)
    xkT = nc.declare_dram_parameter("xkT", [128, SB, 4, 512], BF16, isOutput=False)
    xvT = nc.declare_dram_parameter("xvT", [128, SB, 4, 512], BF16, isOutput=False)
    wqT = nc.declare_dram_parameter("wqT", [128, 4, E], BF16, isOutput=False)
    wkT = nc.declare_dram_parameter("wkT", [128, 4, E], BF16, isOutput=False)
    wvT = nc.declare_dram_parameter("wvT", [128, 4, E], BF16, isOutput=False)
    woT = nc.declare_dram_parameter("woT", [E, D], BF16, isOutput=False)
    bqs = nc.declare_dram_parameter("bqs", [E], F32, isOutput=False)
    bks = nc.declare_dram_parameter("bks", [E], F32, isOutput=False)
    bvb = nc.declare_dram_parameter("bvb", [128, E], F32, isOutput=False)
    e4d = nc.declare_dram_parameter("e4d", [4, 256], F32, isOutput=False)
    yT = nc.declare_dram_parameter("yT", [D, S], F32, isOutput=True)

    Exp = mybir.ActivationFunctionType.Exp

    with SplitDrainTileContext(nc) as tc:
        with tc.sbuf_pool(name="persist", bufs=1) as P:
            QT = P.tile([128, 2, S], BF16)    # e-tiles x queries
            KTt = P.tile([128, 2, S], BF16)
            VA = P.tile([128, KT, 4 * 65], BF16)  # [V_h | ones] per head
            OT = P.tile([128, 2, S], BF16)
            WOT = P.tile([128, 2, D], BF16)
            BQ = P.tile([128, 2], F32)
            BK = P.tile([128, 2], F32)
            BVB = P.tile([128, E], F32)
            # E4[k, j*64+m] = (k==j): selector that broadcasts row j of a
            # [4, 512] tile across 64 partitions via a K=4 matmul.
            E4 = P.tile([4, 256], mybir.dt.float32r)
            E4F = P.tile([4, 256], F32)
            XQT = P.tile([128, SB, 4, 512], BF16)
            XKT = P.tile([128, SB, 4, 512], BF16)
            XVT = P.tile([128, SB, 4, 512], BF16)
            WQ = P.tile([128, 4, E], BF16)
            WK = P.tile([128, 4, E], BF16)
            WVs = P.tile([128, 4, E], BF16)

            # softmax-denominator ones columns of V_aug
            for kt in range(KT):
                va_h = VA[:, kt, :].rearrange("p (h c) -> p h c", c=65)
                nc.vector.memset(va_h[:, :, 64:65], 1.0)
            nc.scalar.dma_start(out=E4F[:, :], in_=e4d[:, :])
            with nc.allow_low_precision(reason="exact 0/1 rounded to fp32r"):
                nc.vector.tensor_copy(E4[:, :], E4F[:, :])

            # ---- input DMAs spread over the three DMA-capable queues: X_K
            # on sync, X_Q on scalar, weights/biases then X_V on gpsimd, so
            # the first projection's inputs land in parallel.
            nc.gpsimd.dma_start(out=WK[:, :, :], in_=wkT[:, :, :])
            nc.gpsimd.dma_start(out=WQ[:, :, :], in_=wqT[:, :, :])
            nc.gpsimd.dma_start(
                out=BQ[:, :], in_=bqs[:].rearrange("(c p) -> p c", p=128)
            )
            nc.gpsimd.dma_start(
                out=BK[:, :], in_=bks[:].rearrange("(c p) -> p c", p=128)
            )
            nc.gpsimd.dma_start(out=WVs[:, :, :], in_=wvT[:, :, :])
            nc.gpsimd.dma_start(out=BVB[:, :], in_=bvb[:, :])
            for sb in range(SB):
                nc.sync.dma_start(out=XKT[:, sb, :, :], in_=xkT[:, sb, :, :])
                nc.scalar.dma_start(out=XQT[:, sb, :, :], in_=xqT[:, sb, :, :])
                nc.gpsimd.dma_start(out=XVT[:, sb, :, :], in_=xvT[:, sb, :, :])
            for et in range(2):
                sl = slice(et * 128, (et + 1) * 128)
                nc.gpsimd.dma_start(out=WOT[:, et, :], in_=woT[sl, :])

            # one shared ring of [128, 512] fp32 PSUM tiles (2 banks) serves
            # the projections, attn@V accumulators, the denominator
            # broadcast, and the output projection; scores get the other 6.
            with tc.psum_pool(name="pp", bufs=2) as PP:
                # Projection emitters. All projections are injected into
                # group 0's score/exp stream so the PE absorbs them during
                # the ACT-bound first attention group instead of serially
                # up front.
                def proj_qk(which, et, sb):
                    xt, wt, out, bias = (
                        (XKT, WK, KTt, BK) if which == "k"
                        else (XQT, WQ, QT, BQ)
                    )
                    ssl = slice(sb * 512, (sb + 1) * 512)
                    ps = PP.tile([128, 512], F32, tag="ps512",
                                 name=f"ps_{which}{et}{sb}")
                    for dt in range(4):
                        nc.tensor.matmul(
                            ps[:, :],
                            lhsT=wt[:, dt, et * 128:(et + 1) * 128],
                            rhs=xt[:, sb, dt, :],
                            start=(dt == 0),
                            stop=(dt == 3),
                        )
                    nc.vector.tensor_scalar_add(
                        out[:, et, ssl], ps[:, :], bias[:, et:et + 1]
                    )

                def proj_v(kt):
                    # V: natural [s, e] + bias, interleaved [V_h | ones]
                    psv = PP.tile([128, 512], F32, tag="ps512",
                                  name=f"psv{kt}")
                    sb, off = divmod(kt * 128, 512)
                    for dt in range(4):
                        nc.tensor.matmul(
                            psv[:, 0:E],
                            lhsT=XVT[:, sb, dt, off:off + 128],
                            rhs=WVs[:, dt, :],
                            start=(dt == 0),
                            stop=(dt == 3),
                        )
                    va_v = VA[:, kt, :].rearrange("p (h c) -> p h c", c=65)
                    psv_h = psv[:, 0:E].rearrange("p (h c) -> p h c", c=64)
                    bvb_h = BVB[:, :].rearrange("p (h c) -> p h c", c=64)
                    nc.vector.tensor_add(
                        va_v[:, :, 0:64], psv_h[:, :, :], bvb_h[:, :, :]
                    )

                # group-0 injection schedule: item index (2*kt+qh) -> work.
                # K0{1,2,3} land just before the scores that need them;
                # V projections and the remaining Q/K groups fill the rest.
                inject = {}
                for i, kt in enumerate(range(KT)):
                    inject.setdefault(i + 1, []).append(
                        (lambda k: lambda: proj_v(k))(kt))
                for sb in (1, 2, 3):
                    inject.setdefault(sb * 8, []).append(
                        (lambda s: lambda: proj_qk("k", 0, s))(sb))
                late = [("k", 1, 0), ("q", 1, 0), ("q", 1, 1), ("k", 1, 1),
                        ("k", 1, 2), ("k", 1, 3), ("q", 0, 2), ("q", 0, 3),
                        ("q", 1, 2), ("q", 1, 3)]
                for i, (w, et_, sb_) in enumerate(late):
                    inject.setdefault(17 + i, []).append(
                        (lambda a, b, c: lambda: proj_qk(a, b, c))(w, et_, sb_))

                # ---- attention: per group (qb, hp): scores+exp stream per
                # (kt, q-half) with attn@V chasing each exp (groups 1-3),
                # then distributed normalization + per-512q output proj.
                with tc.sbuf_pool(name="ptp", bufs=40) as PTP, \
                     tc.sbuf_pool(name="nrm", bufs=4) as NRM, \
                     tc.sbuf_pool(name="yo", bufs=2) as YO, \
                     tc.psum_pool(name="scp", bufs=2) as SCP:
                    for g, (qb, hp) in enumerate([(0, 0), (0, 1),
                                                  (1, 0), (1, 1)]):
                        q0 = qb * 1024
                        et = hp
                        chase = g > 0   # g0's PSUM ring is busy with proj
                        pts = {}
                        opst = {}
                        ous = {}
                        sums = NRM.tile([4, 512], F32, tag="sums", bufs=2,
                                        name=f"sums{g}")

                        def attnv(j2, kt):
                            hh, sq = divmod(j2, 2)
                            h = hp * 2 + hh
                            if kt == 0:
                                opst[j2] = PP.tile([128, 512], F32,
                                                   tag="ps512",
                                                   name=f"ops{g}_{j2}")
                            nc.tensor.matmul(
                                opst[j2][0:65, :],
                                lhsT=VA[:, kt, h * 65:(h + 1) * 65],
                                rhs=pts[sq, kt][:, hh * 512:(hh + 1) * 512],
                                start=(kt == 0),
                                stop=(kt == KT - 1),
                            )

                        def finish_j(j2):
                            ou = NRM.tile([65, 512], F32, tag="ou", bufs=5,
                                          name=f"ou{g}_{j2}")
                            nc.vector.tensor_copy(ou[:, :],
                                                  opst[j2][0:65, :])
                            nc.sync.dma_start(out=sums[j2:j2 + 1, :],
                                              in_=ou[64:65, :])
                            ous[j2] = ou

                        if g == 0:
                            proj_qk("k", 0, 0)
                            proj_qk("q", 0, 0)
                            proj_qk("q", 0, 1)

                        for kt in range(KT):
                            ksl = slice(kt * 128, (kt + 1) * 128)
                            for qh in range(2):
                                if g == 0:
                                    for fn in inject.get(2 * kt + qh, []):
                                        fn()
                                qsl = slice(q0 + qh * 512,
                                            q0 + qh * 512 + 512)
                                with tc.high_priority(offset=300):
                                    sc = SCP.tile([128, 1024], F32,
                                                  tag="sc")
                                    # two K=64 heads in distinct PE row
                                    # groups -> concurrent; outputs land in
                                    # the tile's two PSUM banks.
                                    for hh in range(2):
                                        hsl = slice(hh * 64, hh * 64 + 64)
                                        nc.tensor.matmul(
                                            sc[:, hh * 512:(hh + 1) * 512],
                                            lhsT=KTt[hsl, et, ksl],
                                            rhs=QT[hsl, et, qsl],
                                            start=True,
                                            stop=True,
                                            tile_position=(hh * 64, 0),
                                        )
                                    pt = PTP.tile([128, 1024], BF16,
                                                  tag="pt")
                                    nc.scalar.activation(
                                        pt[:, :], sc[:, :], Exp, scale=SCALE,
                                    )
                                    pts[qh, kt] = pt
                                if chase:
                                    # j2 = hh*2 + sq with sq == qh
                                    attnv(qh, kt)
                                    attnv(2 + qh, kt)
                        if chase:
                            for j2 in range(4):
                                finish_j(j2)
                        else:
                            for j2 in range(4):
                                for kt in range(KT):
                                    attnv(j2, kt)
                                finish_j(j2)
                        rcb = NRM.tile([4, 512], mybir.dt.float32r,
                                       tag="rcb", bufs=2, name=f"rcb{g}")
                        with nc.allow_low_precision(
                            reason="softmax 1/denom rounded to fp32r for "
                            "the selector-matmul broadcast"
                        ):
                            nc.vector.reciprocal(rcb[:, :], sums[:, :])

                        def norm_j(j2):
                            hh, sq = divmod(j2, 2)
                            hsl = slice(hh * 64, hh * 64 + 64)
                            ssl = slice(q0 + sq * 512, q0 + sq * 512 + 512)
                            bc = PP.tile([128, 512], F32, tag="ps512",
                                         name=f"bc{g}_{j2}")
                            nc.tensor.matmul(
                                bc[0:64, :],
                                lhsT=E4[:, j2 * 64:(j2 + 1) * 64],
                                rhs=rcb[:, :],
                                start=True,
                                stop=True,
                            )
                            nc.vector.tensor_mul(
                                OT[hsl, hp, ssl], ous[j2][0:64, :],
                                bc[0:64, :]
                            )

                        def outproj(sq):
                            ssl = slice(q0 + sq * 512, q0 + sq * 512 + 512)
                            for fc in range(4):
                                yp = PP.tile([128, 512], F32, tag="ps512",
                                             name=f"yp{g}_{sq}{fc}")
                                for oe in range(2):
                                    nc.tensor.matmul(
                                        yp[:, :],
                                        lhsT=WOT[:, oe,
                                                 fc * 128:(fc + 1) * 128],
                                        rhs=OT[:, oe, ssl],
                                        start=(oe == 0),
                                        stop=(oe == 1),
                                    )
                                ys = YO.tile([128, 512], F32, tag="ys")
                                nc.vector.tensor_copy(ys[:, :], yp[:, :])
                                nc.sync.dma_start(
                                    out=yT[fc * 128:(fc + 1) * 128, ssl],
                                    in_=ys[:, :],
                                )

                        if hp == 0:
                            for j2 in range(4):
                                norm_j(j2)
                        else:
                            # per 512-q slice: normalize both heads of the
                            # slice, then immediately project it out
                            for sq in range(2):
                                norm_j(sq)        # (hh0, sq)
                                norm_j(2 + sq)    # (hh1, sq)
                                outproj(sq)

    if sanitize:
        sanitize_waits(nc)
    return nc


def _perm_xt(x):
    # (S, D) -> x^T laid out [128, sb, dt, 512]: partition p, block (sb, dt)
    # = row dt*128+p of x^T, columns sb*512:(sb+1)*512
    xt = x.T.astype(NP_BF16)                      # (512, S)
    return np.ascontiguousarray(
        xt.reshape(4, 128, SB, 512).transpose(1, 2, 0, 3)
    )


def _perm_w(w):
    # (E, D) slice of torch weight -> W^T laid out [128, dt, E]
    wt = w.T.astype(NP_BF16)                      # (D, E)
    return np.ascontiguousarray(wt.reshape(4, 128, E).transpose(1, 0, 2))


def _e4():
    e = np.zeros((4, 256), dtype=np.float32)
    for j in range(4):
        e[j, j * 64:(j + 1) * 64] = 1.0
    return e


def make_in_maps(query, key, value, Wq, bq, Wk, bk, Wv, bv, Wo, bo):
    in_maps = []
    for c in range(NCORES):
        b, g = divmod(c, 2)
        eo = g * E
        esl = slice(eo, eo + E)
        in_maps.append({
            "xqT": _perm_xt(query[b]),
            "xkT": _perm_xt(key[b]),
            "xvT": _perm_xt(value[b]),
            "wqT": _perm_w(Wq[esl, :]),
            "wkT": _perm_w(Wk[esl, :]),
            "wvT": _perm_w(Wv[esl, :]),
            "woT": Wo[:, esl].T.astype(NP_BF16),
            "bqs": np.ascontiguousarray(bq[esl], dtype=np.float32),
            "bks": np.ascontiguousarray(bk[esl], dtype=np.float32),
            "bvb": np.ascontiguousarray(
                np.broadcast_to(bv[esl], (128, E)), dtype=np.float32
            ),
            "e4d": _e4(),
        })
    return in_maps


def gather(results, bo):
    out = np.empty((B, S, D), dtype=np.float32)
    for b in range(B):
        yt = results[2 * b]["yT"] + results[2 * b + 1]["yT"]
        out[b] = yt.T + np.asarray(bo, dtype=np.float32)
    return out


_NC = None


def kernel(query, key, value, Wq, bq, Wk, bk, Wv, bv, Wo, bo, **run_kwargs):
    global _NC
    if _NC is None:
        _NC = build_nc()
    args = [np.asarray(a) for a in
            (query, key, value, Wq, bq, Wk, bk, Wv, bv, Wo, bo)]
    in_maps = make_in_maps(*args)
    res = run_bass_kernel_spmd(_NC, in_maps, list(range(NCORES)), **run_kwargs)
    out = gather(res.results, args[10])
    if run_kwargs:
        return out, res
    return out


# revision 35
# speedup vs baseline: 1.3790x; 1.0275x over previous
"""Multi-head attention (B=4, S=2048, D=512, H=8) on 8 trn2 NeuronCores.

Sharding: core c handles batch b=c//2, head-group g=c%2 (4 heads, 256 of the
512 projection dims). Each core runs the full fused pipeline for its four
heads - QKV projection, scores^T = K_h Q_h^T, exp (softmax numerator),
attn @ V with a folded ones-column producing the softmax denominators,
normalization, and its partial output projection y^T = Wo_slice^T.T @ O^T.
The host sums the two partial y^T per batch and adds the output bias.

Key performance structure (vs the v1 kernel):
- Score matmuls for the two heads of an e-tile are issued back-to-back with
  explicit tile_position (0,0)/(64,0): K=64 row-tiled matmuls in distinct
  row groups execute concurrently in the PE array, and their outputs land in
  different PSUM banks of one shared [128,1024] tile (one exp per tile).
- Inputs arrive s-block-major ([128, sb, dt, 512]) so each 512-column
  projection group depends on a single 512KB DMA; attention starts as soon
  as the first blocks land instead of after the full input load.
- Softmax normalization is fully distributed: per (head, 512q) slice, a
  single-pass reciprocal_approx_fast on the denominator row feeds a gpsimd
  partition_broadcast, then one DVE multiply writes normalized O^T. No
  cross-head gather, no batched reciprocal, no selector matmuls: keeps the
  PE streaming gap-free (HAM re-throttles the PE clock to 1.2GHz after idle
  windows, so PE gaps cost double).
"""

import re

import numpy as np
import ml_dtypes

import concourse.bass as bass
import concourse.mybir as mybir
from concourse.bass_utils import run_bass_kernel_spmd
from concourse.tile import ScopedClock, TileContext, VectorClock

BF16 = mybir.dt.bfloat16
F32 = mybir.dt.float32
NP_BF16 = ml_dtypes.bfloat16

B, S, D, H, DK = 4, 2048, 512, 8, 64
SCALE = float(1.0 / (np.float32(np.sqrt(DK)) + 1e-8))
E = 256          # head dims per core (4 heads)
NCORES = 8
KT = S // 128    # 16 key tiles of 128
QB = 2           # q blocks of 1024
SB = S // 512    # 4 s-blocks of 512


# ---------------------------------------------------------------------------
# walrus in this container rejects >1 sync-wait command per instruction;
# split the Tile tail drain and hoist excess mid-kernel waits onto NoOps.
# ---------------------------------------------------------------------------

def _clock_entries(vc):
    nums = [int(s) for s in re.findall(r"-?\d+", repr(vc))]
    return [(i, n) for i, n in enumerate(nums) if n > 0]


class SplitDrainTileContext(TileContext):
    def _drain_and_barrier(self, tick_clock, wait_clock):
        nc = self.nc
        for proc, tick in _clock_entries(tick_clock.global_clock):
            vc = VectorClock()
            vc.require_at_least(proc, tick)
            carrier = nc.sync.nop()
            wait_clock.add_sem_waits(carrier.ins, ScopedClock({None: vc}))
        nc.sync.drain()
        nc.all_engine_barrier()
        assert self.sems is not None
        popped = nc._tile_sem_poison_stack.pop()
        assert popped is self._sem_poison
        nc.clear_and_free_semaphores(list(self.sems.allocated().values()))
        nc.all_engine_barrier()


def sanitize_waits(nc, max_waits: int = 1):
    n_split = 0
    for fn in nc.m.functions:
        for bb in fn.blocks:
            new_insts = []
            for inst in bb.instructions:
                si = inst.sync_info
                waits = list(si.on_wait) if si and si.on_wait else []
                if len(waits) > max_waits:
                    keep = waits[-max_waits:]
                    excess = waits[:-max_waits]
                    for i in range(0, len(excess), max_waits):
                        nop = mybir.InstNoOp(
                            name=nc.get_next_instruction_name(), ins=[], outs=[]
                        )
                        nop.engine = inst.engine
                        nop.sync_info = mybir.SyncInfo(
                            on_wait=excess[i : i + max_waits], on_update=[]
                        )
                        new_insts.append(nop)
                    inst.sync_info = mybir.SyncInfo(
                        on_wait=keep, on_update=si.on_update
                    )
                    n_split += 1
                new_insts.append(inst)
            bb.instructions[:] = new_insts
    return n_split


# ---------------------------------------------------------------------------
# kernel builder (one SPMD program; per-core data differs only in in_maps)
# ---------------------------------------------------------------------------

def build_nc(sanitize=True):
    nc = bass.Bass("TRN2", target_bir_lowering=False, debug=False,
                   num_devices=NCORES)

    # x^T tensors arrive host-permuted as [128, sb, dt, 512]: partition p,
    # block (sb, dt) holds row dt*128+p, columns sb*512... of x^T. One DMA
    # per s-block moves 4KB contiguous per partition (near line rate) and is
    # the single dependency for that block's projection group.
    xqT = nc.declare_dram_parameter("xqT", [128, SB, 4, 512], BF16, isOutput=False)
    xkT = nc.declare_dram_parameter("xkT", [128, SB, 4, 512], BF16, isOutput=False)
    xvT = nc.declare_dram_parameter("xvT", [128, SB, 4, 512], BF16, isOutput=False)
    wqT = nc.declare_dram_parameter("wqT", [128, 4, E], BF16, isOutput=False)
    wkT = nc.declare_dram_parameter("wkT", [128, 4, E], BF16, isOutput=False)
    wvT = nc.declare_dram_parameter("wvT", [128, 4, E], BF16, isOutput=False)
    woT = nc.declare_dram_parameter("woT", [E, D], BF16, isOutput=False)
    bqs = nc.declare_dram_parameter("bqs", [E], F32, isOutput=False)
    bks = nc.declare_dram_parameter("bks", [E], F32, isOutput=False)
    bvb = nc.declare_dram_parameter("bvb", [128, E], F32, isOutput=False)
    e4d = nc.declare_dram_parameter("e4d", [4, 256], F32, isOutput=False)
    yT = nc.declare_dram_parameter("yT", [D, S], F32, isOutput=True)

    Exp = mybir.ActivationFunctionType.Exp

    with SplitDrainTileContext(nc) as tc:
        with tc.sbuf_pool(name="persist", bufs=1) as P:
            QT = P.tile([128, 2, S], BF16)    # e-tiles x queries
            KTt = P.tile([128, 2, S], BF16)
            VA = P.tile([128, KT, 4 * 65], BF16)  # [V_h | ones] per head
            OT = P.tile([128, 2, S], BF16)
            WOT = P.tile([128, 2, D], BF16)
            BQ = P.tile([128, 2], F32)
            BK = P.tile([128, 2], F32)
            BVB = P.tile([128, E], F32)
            # E4[k, j*64+m] = (k==j): selector that broadcasts row j of a
            # [4, 512] tile across 64 partitions via a K=4 matmul.
            E4 = P.tile([4, 256], mybir.dt.float32r)
            E4F = P.tile([4, 256], F32)
            XQT = P.tile([128, SB, 4, 512], BF16)
            XKT = P.tile([128, SB, 4, 512], BF16)
            XVT = P.tile([128, SB, 4, 512], BF16)
            WQ = P.tile([128, 4, E], BF16)
            WK = P.tile([128, 4, E], BF16)
            WVs = P.tile([128, 4, E], BF16)

            # softmax-denominator ones columns of V_aug
            for kt in range(KT):
                va_h = VA[:, kt, :].rearrange("p (h c) -> p h c", c=65)
                nc.vector.memset(va_h[:, :, 64:65], 1.0)
            nc.scalar.dma_start(out=E4F[:, :], in_=e4d[:, :])
            with nc.allow_low_precision(reason="exact 0/1 rounded to fp32r"):
                nc.vector.tensor_copy(E4[:, :], E4F[:, :])

            # ---- input DMAs spread over the three DMA-capable queues: WK +
            # X_K on sync, WQ/biases + X_Q on scalar, WV + X_V on gpsimd,
            # so the first projection's inputs land in parallel.
            nc.sync.dma_start(out=WK[:, :, :], in_=wkT[:, :, :])
            nc.scalar.dma_start(out=WQ[:, :, :], in_=wqT[:, :, :])
            nc.gpsimd.dma_start(out=WVs[:, :, :], in_=wvT[:, :, :])
            nc.gpsimd.dma_start(out=BVB[:, :], in_=bvb[:, :])
            nc.sync.dma_start(out=XKT[:, 0, :, :], in_=xkT[:, 0, :, :])
            nc.scalar.dma_start(out=XQT[:, 0, :, :], in_=xqT[:, 0, :, :])
            nc.gpsimd.dma_start(out=XVT[:, 0, :, :], in_=xvT[:, 0, :, :])
            nc.scalar.dma_start(
                out=BQ[:, :], in_=bqs[:].rearrange("(c p) -> p c", p=128)
            )
            nc.scalar.dma_start(
                out=BK[:, :], in_=bks[:].rearrange("(c p) -> p c", p=128)
            )
            for sb in range(1, SB):
                nc.sync.dma_start(out=XKT[:, sb, :, :], in_=xkT[:, sb, :, :])
                nc.scalar.dma_start(out=XQT[:, sb, :, :], in_=xqT[:, sb, :, :])
                nc.gpsimd.dma_start(out=XVT[:, sb, :, :], in_=xvT[:, sb, :, :])
            for et in range(2):
                sl = slice(et * 128, (et + 1) * 128)
                nc.gpsimd.dma_start(out=WOT[:, et, :], in_=woT[sl, :])

            # one shared ring of [128, 512] fp32 PSUM tiles (2 banks) serves
            # the projections, attn@V accumulators, the denominator
            # broadcast, and the output projection; scores get the other 6.
            with tc.psum_pool(name="pp", bufs=2) as PP:
                # Projection emitters. All projections are injected into
                # group 0's score/exp stream so the PE absorbs them during
                # the ACT-bound first attention group instead of serially
                # up front.
                def proj_qk(which, et, sb):
                    xt, wt, out, bias = (
                        (XKT, WK, KTt, BK) if which == "k"
                        else (XQT, WQ, QT, BQ)
                    )
                    ssl = slice(sb * 512, (sb + 1) * 512)
                    ps = PP.tile([128, 512], F32, tag="ps512",
                                 name=f"ps_{which}{et}{sb}")
                    for dt in range(4):
                        nc.tensor.matmul(
                            ps[:, :],
                            lhsT=wt[:, dt, et * 128:(et + 1) * 128],
                            rhs=xt[:, sb, dt, :],
                            start=(dt == 0),
                            stop=(dt == 3),
                        )
                    nc.vector.tensor_scalar_add(
                        out[:, et, ssl], ps[:, :], bias[:, et:et + 1]
                    )

                def proj_v(kt):
                    # V: natural [s, e] + bias, interleaved [V_h | ones]
                    psv = PP.tile([128, 512], F32, tag="ps512",
                                  name=f"psv{kt}")
                    sb, off = divmod(kt * 128, 512)
                    for dt in range(4):
                        nc.tensor.matmul(
                            psv[:, 0:E],
                            lhsT=XVT[:, sb, dt, off:off + 128],
                            rhs=WVs[:, dt, :],
                            start=(dt == 0),
                            stop=(dt == 3),
                        )
                    va_v = VA[:, kt, :].rearrange("p (h c) -> p h c", c=65)
                    psv_h = psv[:, 0:E].rearrange("p (h c) -> p h c", c=64)
                    bvb_h = BVB[:, :].rearrange("p (h c) -> p h c", c=64)
                    nc.vector.tensor_add(
                        va_v[:, :, 0:64], psv_h[:, :, :], bvb_h[:, :, :]
                    )

                # group-0 injection schedule: item index (2*kt+qh) -> work.
                # K0{1,2,3} land just before the scores that need them; V
                # projections trail their s-block DMAs; the remaining Q/K
                # groups fill the back half.
                inject = {}
                for kt in range(KT):
                    inject.setdefault(4 + kt, []).append(
                        (lambda k: lambda: proj_v(k))(kt))
                for sb in (1, 2, 3):
                    inject.setdefault(sb * 8, []).append(
                        (lambda s: lambda: proj_qk("k", 0, s))(sb))
                late = [("k", 1, 0), ("q", 1, 0), ("q", 1, 1), ("k", 1, 1),
                        ("k", 1, 2), ("k", 1, 3), ("q", 0, 2), ("q", 0, 3),
                        ("q", 1, 2), ("q", 1, 3)]
                for i, (w, et_, sb_) in enumerate(late):
                    inject.setdefault(20 + i, []).append(
                        (lambda a, b, c: lambda: proj_qk(a, b, c))(w, et_, sb_))

                # ---- attention: per group (qb, hp): scores+exp stream per
                # (kt, q-half) with attn@V chasing each exp (groups 1-3),
                # then distributed normalization + per-512q output proj.
                with tc.sbuf_pool(name="ptp", bufs=40) as PTP, \
                     tc.sbuf_pool(name="nrm", bufs=4) as NRM, \
                     tc.sbuf_pool(name="yo", bufs=2) as YO, \
                     tc.psum_pool(name="scp", bufs=2) as SCP:
                    for g, (qb, hp) in enumerate([(0, 0), (0, 1),
                                                  (1, 0), (1, 1)]):
                        q0 = qb * 1024
                        et = hp
                        chase = g > 0   # g0's PSUM ring is busy with proj
                        pts = {}
                        opst = {}
                        ous = {}
                        rcbs = {}
                        # per-512q sums tiles: rows (hh0, hh1) of one slice,
                        # so sq0's reciprocal runs while sq1 still streams
                        sums = {
                            sq: NRM.tile([2, 512], F32, tag="sums", bufs=4,
                                         name=f"sums{g}_{sq}")
                            for sq in range(2)
                        }

                        def attnv(j2, kt):
                            hh, sq = divmod(j2, 2)
                            h = hp * 2 + hh
                            if kt == 0:
                                opst[j2] = PP.tile([128, 512], F32,
                                                   tag="ps512",
                                                   name=f"ops{g}_{j2}")
                            nc.tensor.matmul(
                                opst[j2][0:65, :],
                                lhsT=VA[:, kt, h * 65:(h + 1) * 65],
                                rhs=pts[sq, kt][:, hh * 512:(hh + 1) * 512],
                                start=(kt == 0),
                                stop=(kt == KT - 1),
                            )

                        def finish_j(j2):
                            hh, sq = divmod(j2, 2)
                            ou = NRM.tile([65, 512], F32, tag="ou", bufs=5,
                                          name=f"ou{g}_{j2}")
                            nc.vector.tensor_copy(ou[:, :],
                                                  opst[j2][0:65, :])
                            nc.sync.dma_start(out=sums[sq][hh:hh + 1, :],
                                              in_=ou[64:65, :])
                            ous[j2] = ou

                        def recip_sq(sq):
                            rcb = NRM.tile([2, 512], mybir.dt.float32r,
                                           tag="rcb", bufs=4,
                                           name=f"rcb{g}_{sq}")
                            with nc.allow_low_precision(
                                reason="softmax 1/denom rounded to fp32r "
                                "for the selector-matmul broadcast"
                            ):
                                nc.vector.reciprocal(rcb[:, :],
                                                     sums[sq][:, :])
                            rcbs[sq] = rcb

                        if g == 0:
                            proj_qk("k", 0, 0)
                            proj_qk("q", 0, 0)
                            proj_qk("q", 0, 1)

                        for kt in range(KT):
                            ksl = slice(kt * 128, (kt + 1) * 128)
                            for qh in range(2):
                                if g == 0:
                                    for fn in inject.get(2 * kt + qh, []):
                                        fn()
                                qsl = slice(q0 + qh * 512,
                                            q0 + qh * 512 + 512)
                                with tc.high_priority(offset=300):
                                    sc = SCP.tile([128, 1024], F32,
                                                  tag="sc")
                                    # two K=64 heads in distinct PE row
                                    # groups -> concurrent; outputs land in
                                    # the tile's two PSUM banks.
                                    for hh in range(2):
                                        hsl = slice(hh * 64, hh * 64 + 64)
                                        nc.tensor.matmul(
                                            sc[:, hh * 512:(hh + 1) * 512],
                                            lhsT=KTt[hsl, et, ksl],
                                            rhs=QT[hsl, et, qsl],
                                            start=True,
                                            stop=True,
                                            tile_position=(hh * 64, 0),
                                        )
                                    pt = PTP.tile([128, 1024], BF16,
                                                  tag="pt")
                                    nc.scalar.activation(
                                        pt[:, :], sc[:, :], Exp, scale=SCALE,
                                    )
                                    pts[qh, kt] = pt
                                if chase:
                                    # j2 = hh*2 + sq with sq == qh
                                    attnv(qh, kt)
                                    attnv(2 + qh, kt)
                                    if kt == KT - 1 and qh == 0:
                                        # sq0 accumulators are complete:
                                        # start its normalization while the
                                        # last sq1 items stream
                                        finish_j(0)
                                        finish_j(2)
                                        recip_sq(0)
                        if chase:
                            finish_j(1)
                            finish_j(3)
                            recip_sq(1)
                        else:
                            for j2 in (0, 2, 1, 3):
                                for kt in range(KT):
                                    attnv(j2, kt)
                                finish_j(j2)
                                if j2 == 2:
                                    recip_sq(0)
                            recip_sq(1)

                        def norm_j(j2):
                            hh, sq = divmod(j2, 2)
                            hsl = slice(hh * 64, hh * 64 + 64)
                            ssl = slice(q0 + sq * 512, q0 + sq * 512 + 512)
                            bc = PP.tile([128, 512], F32, tag="ps512",
                                         name=f"bc{g}_{j2}")
                            nc.tensor.matmul(
                                bc[0:64, :],
                                lhsT=E4[0:2, hh * 64:(hh + 1) * 64],
                                rhs=rcbs[sq][:, :],
                                start=True,
                                stop=True,
                            )
                            nc.vector.tensor_mul(
                                OT[hsl, hp, ssl], ous[j2][0:64, :],
                                bc[0:64, :]
                            )

                        def outproj(sq):
                            ssl = slice(q0 + sq * 512, q0 + sq * 512 + 512)
                            for fc in range(4):
                                yp = PP.tile([128, 512], F32, tag="ps512",
                                             name=f"yp{g}_{sq}{fc}")
                                for oe in range(2):
                                    nc.tensor.matmul(
                                        yp[:, :],
                                        lhsT=WOT[:, oe,
                                                 fc * 128:(fc + 1) * 128],
                                        rhs=OT[:, oe, ssl],
                                        start=(oe == 0),
                                        stop=(oe == 1),
                                    )
                                ys = YO.tile([128, 512], F32, tag="ys")
                                nc.vector.tensor_copy(ys[:, :], yp[:, :])
                                nc.sync.dma_start(
                                    out=yT[fc * 128:(fc + 1) * 128, ssl],
                                    in_=ys[:, :],
                                )

                        if hp == 0:
                            for j2 in (0, 2, 1, 3):
                                norm_j(j2)
                        else:
                            # per 512-q slice: normalize both heads of the
                            # slice, then immediately project it out
                            for sq in range(2):
                                norm_j(sq)        # (hh0, sq)
                                norm_j(2 + sq)    # (hh1, sq)
                                outproj(sq)

    if sanitize:
        sanitize_waits(nc)
    return nc


def _perm_xt(x):
    # (S, D) -> x^T laid out [128, sb, dt, 512]: partition p, block (sb, dt)
    # = row dt*128+p of x^T, columns sb*512:(sb+1)*512
    xt = x.T.astype(NP_BF16)                      # (512, S)
    return np.ascontiguousarray(
        xt.reshape(4, 128, SB, 512).transpose(1, 2, 0, 3)
    )


def _perm_w(w):
    # (E, D) slice of torch weight -> W^T laid out [128, dt, E]
    wt = w.T.astype(NP_BF16)                      # (D, E)
    return np.ascontiguousarray(wt.reshape(4, 128, E).transpose(1, 0, 2))


def _e4():
    e = np.zeros((4, 256), dtype=np.float32)
    for j in range(4):
        e[j, j * 64:(j + 1) * 64] = 1.0
    return e


def make_in_maps(query, key, value, Wq, bq, Wk, bk, Wv, bv, Wo, bo):
    in_maps = []
    for c in range(NCORES):
        b, g = divmod(c, 2)
        eo = g * E
        esl = slice(eo, eo + E)
        in_maps.append({
            "xqT": _perm_xt(query[b]),
            "xkT": _perm_xt(key[b]),
            "xvT": _perm_xt(value[b]),
            "wqT": _perm_w(Wq[esl, :]),
            "wkT": _perm_w(Wk[esl, :]),
            "wvT": _perm_w(Wv[esl, :]),
            "woT": Wo[:, esl].T.astype(NP_BF16),
            "bqs": np.ascontiguousarray(bq[esl], dtype=np.float32),
            "bks": np.ascontiguousarray(bk[esl], dtype=np.float32),
            "bvb": np.ascontiguousarray(
                np.broadcast_to(bv[esl], (128, E)), dtype=np.float32
            ),
            "e4d": _e4(),
        })
    return in_maps


def gather(results, bo):
    out = np.empty((B, S, D), dtype=np.float32)
    for b in range(B):
        yt = results[2 * b]["yT"] + results[2 * b + 1]["yT"]
        out[b] = yt.T + np.asarray(bo, dtype=np.float32)
    return out


_NC = None


def kernel(query, key, value, Wq, bq, Wk, bk, Wv, bv, Wo, bo, **run_kwargs):
    global _NC
    if _NC is None:
        _NC = build_nc()
    args = [np.asarray(a) for a in
            (query, key, value, Wq, bq, Wk, bk, Wv, bv, Wo, bo)]
    in_maps = make_in_maps(*args)
    res = run_bass_kernel_spmd(_NC, in_maps, list(range(NCORES)), **run_kwargs)
    out = gather(res.results, args[10])
    if run_kwargs:
        return out, res
    return out


# revision 38
# speedup vs baseline: 1.4396x; 1.0440x over previous
"""Multi-head attention (B=4, S=2048, D=512, H=8) on 8 trn2 NeuronCores.

Sharding: core c handles batch b=c//2, head-group g=c%2 (4 heads, 256 of the
512 projection dims). Each core runs the full fused pipeline for its four
heads - QKV projection, scores^T = K_h Q_h^T, exp (softmax numerator),
attn @ V with a folded ones-column producing the softmax denominators,
normalization, and its partial output projection y^T = Wo_slice^T.T @ O^T.
The host sums the two partial y^T per batch and adds the output bias.

Key performance structure (vs the v1 kernel):
- Score matmuls for the two heads of an e-tile are issued back-to-back with
  explicit tile_position (0,0)/(64,0): K=64 row-tiled matmuls in distinct
  row groups execute concurrently in the PE array, and their outputs land in
  different PSUM banks of one shared [128,1024] tile (one exp per tile).
- Inputs arrive s-block-major ([128, sb, dt, 512]) so each 512-column
  projection group depends on a single 512KB DMA; attention starts as soon
  as the first blocks land instead of after the full input load.
- Softmax normalization is fully distributed: per (head, 512q) slice, a
  single-pass reciprocal_approx_fast on the denominator row feeds a gpsimd
  partition_broadcast, then one DVE multiply writes normalized O^T. No
  cross-head gather, no batched reciprocal, no selector matmuls: keeps the
  PE streaming gap-free (HAM re-throttles the PE clock to 1.2GHz after idle
  windows, so PE gaps cost double).
"""

import re

import numpy as np
import ml_dtypes

import concourse.bass as bass
import concourse.mybir as mybir
from concourse.bass_utils import run_bass_kernel_spmd
from concourse.tile import ScopedClock, TileContext, VectorClock

BF16 = mybir.dt.bfloat16
F32 = mybir.dt.float32
NP_BF16 = ml_dtypes.bfloat16

B, S, D, H, DK = 4, 2048, 512, 8, 64
SCALE = float(1.0 / (np.float32(np.sqrt(DK)) + 1e-8))
E = 256          # head dims per core (4 heads)
NCORES = 8
KT = S // 128    # 16 key tiles of 128
QB = 2           # q blocks of 1024
SB = S // 512    # 4 s-blocks of 512


# ---------------------------------------------------------------------------
# walrus in this container rejects >1 sync-wait command per instruction;
# split the Tile tail drain and hoist excess mid-kernel waits onto NoOps.
# ---------------------------------------------------------------------------

def _clock_entries(vc):
    nums = [int(s) for s in re.findall(r"-?\d+", repr(vc))]
    return [(i, n) for i, n in enumerate(nums) if n > 0]


class SplitDrainTileContext(TileContext):
    def _drain_and_barrier(self, tick_clock, wait_clock):
        nc = self.nc
        for proc, tick in _clock_entries(tick_clock.global_clock):
            vc = VectorClock()
            vc.require_at_least(proc, tick)
            carrier = nc.sync.nop()
            wait_clock.add_sem_waits(carrier.ins, ScopedClock({None: vc}))
        nc.sync.drain()
        nc.all_engine_barrier()
        assert self.sems is not None
        popped = nc._tile_sem_poison_stack.pop()
        assert popped is self._sem_poison
        nc.clear_and_free_semaphores(list(self.sems.allocated().values()))
        nc.all_engine_barrier()


def sanitize_waits(nc, max_waits: int = 1):
    n_split = 0
    for fn in nc.m.functions:
        for bb in fn.blocks:
            new_insts = []
            for inst in bb.instructions:
                si = inst.sync_info
                waits = list(si.on_wait) if si and si.on_wait else []
                if len(waits) > max_waits:
                    keep = waits[-max_waits:]
                    excess = waits[:-max_waits]
                    for i in range(0, len(excess), max_waits):
                        nop = mybir.InstNoOp(
                            name=nc.get_next_instruction_name(), ins=[], outs=[]
                        )
                        nop.engine = inst.engine
                        nop.sync_info = mybir.SyncInfo(
                            on_wait=excess[i : i + max_waits], on_update=[]
                        )
                        new_insts.append(nop)
                    inst.sync_info = mybir.SyncInfo(
                        on_wait=keep, on_update=si.on_update
                    )
                    n_split += 1
                new_insts.append(inst)
            bb.instructions[:] = new_insts
    return n_split


# ---------------------------------------------------------------------------
# kernel builder (one SPMD program; per-core data differs only in in_maps)
# ---------------------------------------------------------------------------

def build_nc(sanitize=True):
    nc = bass.Bass("TRN2", target_bir_lowering=False, debug=False,
                   num_devices=NCORES)

    # x^T tensors arrive host-permuted as [128, sb, dt, 512]: partition p,
    # block (sb, dt) holds row dt*128+p, columns sb*512... of x^T. One DMA
    # per s-block moves 4KB contiguous per partition (near line rate) and is
    # the single dependency for that block's projection group.
    xqT = nc.declare_dram_parameter("xqT", [128, SB, 4, 512], BF16, isOutput=False)
    xkT = nc.declare_dram_parameter("xkT", [128, SB, 4, 512], BF16, isOutput=False)
    xvT = nc.declare_dram_parameter("xvT", [128, SB, 4, 512], BF16, isOutput=False)
    wqT = nc.declare_dram_parameter("wqT", [128, 4, E], BF16, isOutput=False)
    wkT = nc.declare_dram_parameter("wkT", [128, 4, E], BF16, isOutput=False)
    wvT = nc.declare_dram_parameter("wvT", [128, 4, E], BF16, isOutput=False)
    woT = nc.declare_dram_parameter("woT", [E, D], BF16, isOutput=False)
    bqs = nc.declare_dram_parameter("bqs", [E], F32, isOutput=False)
    bks = nc.declare_dram_parameter("bks", [E], F32, isOutput=False)
    bvb = nc.declare_dram_parameter("bvb", [128, E], F32, isOutput=False)
    e4d = nc.declare_dram_parameter("e4d", [4, 256], F32, isOutput=False)
    yT = nc.declare_dram_parameter("yT", [D, S], F32, isOutput=True)

    Exp = mybir.ActivationFunctionType.Exp

    with SplitDrainTileContext(nc) as tc:
        with tc.sbuf_pool(name="persist", bufs=1) as P:
            QT = P.tile([128, 2, S], BF16)    # e-tiles x queries
            KTt = P.tile([128, 2, S], BF16)
            VA = P.tile([128, KT, 4 * 65], BF16)  # [V_h | ones] per head
            OT = P.tile([128, 2, S], BF16)
            WOT = P.tile([128, 2, D], BF16)
            BQ = P.tile([128, 2], F32)
            BK = P.tile([128, 2], F32)
            BVB = P.tile([128, E], F32)
            # E4[k, j*64+m] = (k==j): selector that broadcasts row j of a
            # [4, 512] tile across 64 partitions via a K=4 matmul.
            E4 = P.tile([4, 256], mybir.dt.float32r)
            E4F = P.tile([4, 256], F32)
            XQT = P.tile([128, SB, 4, 512], BF16)
            XKT = P.tile([128, SB, 4, 512], BF16)
            XVT = P.tile([128, SB, 4, 512], BF16)
            WQ = P.tile([128, 4, E], BF16)
            WK = P.tile([128, 4, E], BF16)
            WVs = P.tile([128, 4, E], BF16)

            # softmax-denominator ones columns of V_aug
            for kt in range(KT):
                va_h = VA[:, kt, :].rearrange("p (h c) -> p h c", c=65)
                nc.vector.memset(va_h[:, :, 64:65], 1.0)
            # preload the Exp table while input DMAs are in flight (the ones
            # column in VA is memset already and makes a harmless input)
            scr = P.tile([1, 1], F32)
            nc.scalar.activation(scr[:, :], VA[0:1, 0, 64:65], Exp)

            # ---- input DMAs spread over the three DMA-capable queues: WK +
            # X_K on sync, WQ + X_Q on scalar, WV/biases + X_V on gpsimd,
            # critical blocks first so the first projections land early.
            nc.sync.dma_start(out=WK[:, :, :], in_=wkT[:, :, :])
            nc.scalar.dma_start(out=WQ[:, :, :], in_=wqT[:, :, :])
            nc.gpsimd.dma_start(out=WVs[:, :, :], in_=wvT[:, :, :])
            nc.sync.dma_start(out=XKT[:, 0, :, :], in_=xkT[:, 0, :, :])
            nc.scalar.dma_start(out=XQT[:, 0, :, :], in_=xqT[:, 0, :, :])
            nc.gpsimd.dma_start(out=XVT[:, 0, :, :], in_=xvT[:, 0, :, :])
            nc.gpsimd.dma_start(out=BVB[:, :], in_=bvb[:, :])
            nc.gpsimd.dma_start(
                out=BQ[:, :], in_=bqs[:].rearrange("(c p) -> p c", p=128)
            )
            nc.gpsimd.dma_start(
                out=BK[:, :], in_=bks[:].rearrange("(c p) -> p c", p=128)
            )
            nc.sync.dma_start(out=XKT[:, 1, :, :], in_=xkT[:, 1, :, :])
            nc.scalar.dma_start(out=XQT[:, 1, :, :], in_=xqT[:, 1, :, :])
            nc.gpsimd.dma_start(out=XVT[:, 1, :, :], in_=xvT[:, 1, :, :])
            nc.scalar.dma_start(out=E4F[:, :], in_=e4d[:, :])
            for sb in range(2, SB):
                nc.sync.dma_start(out=XKT[:, sb, :, :], in_=xkT[:, sb, :, :])
                nc.scalar.dma_start(out=XQT[:, sb, :, :], in_=xqT[:, sb, :, :])
                nc.gpsimd.dma_start(out=XVT[:, sb, :, :], in_=xvT[:, sb, :, :])
            for et in range(2):
                sl = slice(et * 128, (et + 1) * 128)
                nc.gpsimd.dma_start(out=WOT[:, et, :], in_=woT[sl, :])
            with nc.allow_low_precision(reason="exact 0/1 rounded to fp32r"):
                nc.vector.tensor_copy(E4[:, :], E4F[:, :])

            # one shared ring of [128, 512] fp32 PSUM tiles (2 banks) serves
            # the projections, attn@V accumulators, the denominator
            # broadcast, and the output projection; scores get the other 6.
            with tc.psum_pool(name="pp", bufs=2) as PP:
                # Projection emitters. All projections are injected into
                # group 0's score/exp stream so the PE absorbs them during
                # the ACT-bound first attention group instead of serially
                # up front.
                def proj_qk(which, et, sb):
                    xt, wt, out, bias = (
                        (XKT, WK, KTt, BK) if which == "k"
                        else (XQT, WQ, QT, BQ)
                    )
                    ssl = slice(sb * 512, (sb + 1) * 512)
                    ps = PP.tile([128, 512], F32, tag="ps512",
                                 name=f"ps_{which}{et}{sb}")
                    for dt in range(4):
                        nc.tensor.matmul(
                            ps[:, :],
                            lhsT=wt[:, dt, et * 128:(et + 1) * 128],
                            rhs=xt[:, sb, dt, :],
                            start=(dt == 0),
                            stop=(dt == 3),
                        )
                    nc.vector.tensor_scalar_add(
                        out[:, et, ssl], ps[:, :], bias[:, et:et + 1]
                    )

                def proj_v(kt):
                    # V: natural [s, e] + bias, interleaved [V_h | ones]
                    psv = PP.tile([128, 512], F32, tag="ps512",
                                  name=f"psv{kt}")
                    sb, off = divmod(kt * 128, 512)
                    for dt in range(4):
                        nc.tensor.matmul(
                            psv[:, 0:E],
                            lhsT=XVT[:, sb, dt, off:off + 128],
                            rhs=WVs[:, dt, :],
                            start=(dt == 0),
                            stop=(dt == 3),
                        )
                    va_v = VA[:, kt, :].rearrange("p (h c) -> p h c", c=65)
                    psv_h = psv[:, 0:E].rearrange("p (h c) -> p h c", c=64)
                    bvb_h = BVB[:, :].rearrange("p (h c) -> p h c", c=64)
                    nc.vector.tensor_add(
                        va_v[:, :, 0:64], psv_h[:, :, :], bvb_h[:, :, :]
                    )

                # group-0 injection schedule: item index (2*kt+qh) -> work.
                # K0{1,2,3} land just before the scores that need them; V
                # projections trail their s-block DMAs; the remaining Q/K
                # groups fill the back half.
                inject = {}
                for kt in range(KT):
                    inject.setdefault(4 + kt, []).append(
                        (lambda k: lambda: proj_v(k))(kt))
                for sb in (1, 2, 3):
                    inject.setdefault(sb * 8, []).append(
                        (lambda s: lambda: proj_qk("k", 0, s))(sb))
                late = [("k", 1, 0), ("q", 1, 0), ("q", 1, 1), ("k", 1, 1),
                        ("k", 1, 2), ("k", 1, 3), ("q", 0, 2), ("q", 0, 3),
                        ("q", 1, 2), ("q", 1, 3)]
                for i, (w, et_, sb_) in enumerate(late):
                    inject.setdefault(20 + i, []).append(
                        (lambda a, b, c: lambda: proj_qk(a, b, c))(w, et_, sb_))

                # ---- attention: per group (qb, hp): scores+exp stream per
                # (kt, q-half) with attn@V chasing each exp (groups 1-3),
                # then distributed normalization + per-512q output proj.
                with tc.sbuf_pool(name="ptp", bufs=40) as PTP, \
                     tc.sbuf_pool(name="nrm", bufs=4) as NRM, \
                     tc.sbuf_pool(name="yo", bufs=2) as YO, \
                     tc.psum_pool(name="scp", bufs=2) as SCP:
                    for g, (qb, hp) in enumerate([(0, 0), (0, 1),
                                                  (1, 0), (1, 1)]):
                        q0 = qb * 1024
                        et = hp
                        chase = g > 0   # g0's PSUM ring is busy with proj
                        pts = {}
                        opst = {}
                        ous = {}
                        rcbs = {}
                        # per-512q sums tiles: rows (hh0, hh1) of one slice,
                        # so sq0's reciprocal runs while sq1 still streams
                        sums = {
                            sq: NRM.tile([2, 512], F32, tag="sums", bufs=4,
                                         name=f"sums{g}_{sq}")
                            for sq in range(2)
                        }

                        def attnv(j2, kt):
                            hh, sq = divmod(j2, 2)
                            h = hp * 2 + hh
                            if kt == 0:
                                opst[j2] = PP.tile([128, 512], F32,
                                                   tag="ps512",
                                                   name=f"ops{g}_{j2}")
                            nc.tensor.matmul(
                                opst[j2][0:65, :],
                                lhsT=VA[:, kt, h * 65:(h + 1) * 65],
                                rhs=pts[sq, kt][:, hh * 512:(hh + 1) * 512],
                                start=(kt == 0),
                                stop=(kt == KT - 1),
                            )

                        def finish_j(j2):
                            hh, sq = divmod(j2, 2)
                            ou = NRM.tile([65, 512], F32, tag="ou", bufs=5,
                                          name=f"ou{g}_{j2}")
                            nc.vector.tensor_copy(ou[:, :],
                                                  opst[j2][0:65, :])
                            nc.sync.dma_start(out=sums[sq][hh:hh + 1, :],
                                              in_=ou[64:65, :])
                            ous[j2] = ou

                        def recip_sq(sq):
                            rcb = NRM.tile([2, 512], mybir.dt.float32r,
                                           tag="rcb", bufs=4,
                                           name=f"rcb{g}_{sq}")
                            with nc.allow_low_precision(
                                reason="softmax 1/denom rounded to fp32r "
                                "for the selector-matmul broadcast"
                            ):
                                nc.vector.reciprocal(rcb[:, :],
                                                     sums[sq][:, :])
                            rcbs[sq] = rcb

                        if g == 0:
                            proj_qk("k", 0, 0)
                            proj_qk("q", 0, 0)
                            proj_qk("q", 0, 1)

                        for kt in range(KT):
                            ksl = slice(kt * 128, (kt + 1) * 128)
                            for qh in range(2):
                                if g == 0:
                                    for fn in inject.get(2 * kt + qh, []):
                                        fn()
                                qsl = slice(q0 + qh * 512,
                                            q0 + qh * 512 + 512)
                                with tc.high_priority(offset=300):
                                    sc = SCP.tile([128, 1024], F32,
                                                  tag="sc")
                                    # two K=64 heads in distinct PE row
                                    # groups -> concurrent; outputs land in
                                    # the tile's two PSUM banks.
                                    for hh in range(2):
                                        hsl = slice(hh * 64, hh * 64 + 64)
                                        nc.tensor.matmul(
                                            sc[:, hh * 512:(hh + 1) * 512],
                                            lhsT=KTt[hsl, et, ksl],
                                            rhs=QT[hsl, et, qsl],
                                            start=True,
                                            stop=True,
                                            tile_position=(hh * 64, 0),
                                        )
                                    pt = PTP.tile([128, 1024], BF16,
                                                  tag="pt")
                                    nc.scalar.activation(
                                        pt[:, :], sc[:, :], Exp, scale=SCALE,
                                    )
                                    pts[qh, kt] = pt
                                if chase:
                                    # j2 = hh*2 + sq with sq == qh
                                    attnv(qh, kt)
                                    attnv(2 + qh, kt)
                                    if kt == KT - 1 and qh == 0:
                                        # sq0 accumulators are complete:
                                        # start its normalization while the
                                        # last sq1 items stream
                                        finish_j(0)
                                        finish_j(2)
                                        recip_sq(0)
                        if chase:
                            finish_j(1)
                            finish_j(3)
                            recip_sq(1)
                        else:
                            for j2 in (0, 2, 1, 3):
                                for kt in range(KT):
                                    attnv(j2, kt)
                                finish_j(j2)
                                if j2 == 2:
                                    recip_sq(0)
                            recip_sq(1)

                        def norm_j(j2):
                            hh, sq = divmod(j2, 2)
                            hsl = slice(hh * 64, hh * 64 + 64)
                            ssl = slice(q0 + sq * 512, q0 + sq * 512 + 512)
                            bc = PP.tile([128, 512], F32, tag="ps512",
                                         name=f"bc{g}_{j2}")
                            nc.tensor.matmul(
                                bc[0:64, :],
                                lhsT=E4[0:2, hh * 64:(hh + 1) * 64],
                                rhs=rcbs[sq][:, :],
                                start=True,
                                stop=True,
                            )
                            nc.vector.tensor_mul(
                                OT[hsl, hp, ssl], ous[j2][0:64, :],
                                bc[0:64, :]
                            )

                        def outproj(sq):
                            ssl = slice(q0 + sq * 512, q0 + sq * 512 + 512)
                            for fc in range(4):
                                yp = PP.tile([128, 512], F32, tag="ps512",
                                             name=f"yp{g}_{sq}{fc}")
                                for oe in range(2):
                                    nc.tensor.matmul(
                                        yp[:, :],
                                        lhsT=WOT[:, oe,
                                                 fc * 128:(fc + 1) * 128],
                                        rhs=OT[:, oe, ssl],
                                        start=(oe == 0),
                                        stop=(oe == 1),
                                    )
                                ys = YO.tile([128, 512], F32, tag="ys")
                                nc.vector.tensor_copy(ys[:, :], yp[:, :])
                                nc.sync.dma_start(
                                    out=yT[fc * 128:(fc + 1) * 128, ssl],
                                    in_=ys[:, :],
                                )

                        if hp == 0:
                            for j2 in (0, 2, 1, 3):
                                norm_j(j2)
                        else:
                            # per 512-q slice: normalize both heads of the
                            # slice, then immediately project it out
                            for sq in range(2):
                                norm_j(sq)        # (hh0, sq)
                                norm_j(2 + sq)    # (hh1, sq)
                                outproj(sq)

    if sanitize:
        sanitize_waits(nc)
    return nc


def _perm_xt(x):
    # (S, D) -> x^T laid out [128, sb, dt, 512]: partition p, block (sb, dt)
    # = row dt*128+p of x^T, columns sb*512:(sb+1)*512
    xt = x.T.astype(NP_BF16)                      # (512, S)
    return np.ascontiguousarray(
        xt.reshape(4, 128, SB, 512).transpose(1, 2, 0, 3)
    )


def _perm_w(w):
    # (E, D) slice of torch weight -> W^T laid out [128, dt, E]
    wt = w.T.astype(NP_BF16)                      # (D, E)
    return np.ascontiguousarray(wt.reshape(4, 128, E).transpose(1, 0, 2))


def _e4():
    e = np.zeros((4, 256), dtype=np.float32)
    for j in range(4):
        e[j, j * 64:(j + 1) * 64] = 1.0
    return e


def make_in_maps(query, key, value, Wq, bq, Wk, bk, Wv, bv, Wo, bo):
    in_maps = []
    for c in range(NCORES):
        b, g = divmod(c, 2)
        eo = g * E
        esl = slice(eo, eo + E)
        in_maps.append({
            "xqT": _perm_xt(query[b]),
            "xkT": _perm_xt(key[b]),
            "xvT": _perm_xt(value[b]),
            "wqT": _perm_w(Wq[esl, :]),
            "wkT": _perm_w(Wk[esl, :]),
            "wvT": _perm_w(Wv[esl, :]),
            "woT": Wo[:, esl].T.astype(NP_BF16),
            "bqs": np.ascontiguousarray(bq[esl], dtype=np.float32),
            "bks": np.ascontiguousarray(bk[esl], dtype=np.float32),
            "bvb": np.ascontiguousarray(
                np.broadcast_to(bv[esl], (128, E)), dtype=np.float32
            ),
            "e4d": _e4(),
        })
    return in_maps


def gather(results, bo):
    out = np.empty((B, S, D), dtype=np.float32)
    for b in range(B):
        yt = results[2 * b]["yT"] + results[2 * b + 1]["yT"]
        out[b] = yt.T + np.asarray(bo, dtype=np.float32)
    return out


_NC = None


def kernel(query, key, value, Wq, bq, Wk, bk, Wv, bv, Wo, bo, **run_kwargs):
    global _NC
    if _NC is None:
        _NC = build_nc()
    args = [np.asarray(a) for a in
            (query, key, value, Wq, bq, Wk, bk, Wv, bv, Wo, bo)]
    in_maps = make_in_maps(*args)
    res = run_bass_kernel_spmd(_NC, in_maps, list(range(NCORES)), **run_kwargs)
    out = gather(res.results, args[10])
    if run_kwargs:
        return out, res
    return out
